# revision 44
# baseline (speedup 1.0000x reference)
"""Causal single-head attention (B=4, S=2048, d=1024) on 8 trn2 NeuronCores.

Sharding: core c -> batch c//2, query-parity c%2. Queries of one batch are
split by even/odd 128-row blocks (interleaved so causal work balances);
every core runs the IDENTICAL program -- the host gathers each core's query
rows into a dense x_qT input, and two per-core [128,512] additive masks
encode the causal boundary.

Score path (weight-fused): scores = (x Wq)(x Wk)^T = x (Wq Wk^T) x^T.
The host folds the two projection weights into M = Wq Wk^T once
(input-independent weight preprocessing), so the kernel computes
t = x_q @ M on its query rows only and contracts t directly against the
fp8 x^T already resident in SBUF -- the K projection disappears
entirely (it was the largest PE block and fully duplicated across each
core pair). Host passes fp8(32*M); the 32x score scale is folded into
the exp. Validated vs f64 reference: rel-max ~1.0e-2 (better than the
q8/k8 route's 1.34e-2 -- one fewer weight-quantization + requant stage).

Mixed precision: t projection + t.x^T scores run as fp8 DoubleRow
matmuls (K=256/instr, 2x PE rate). E (exp output), E^T transposes, AV
and the out-projection run in bf16 (fp8 AV/out-proj tested numerically:
3-4e-2 rel-max, over the 2e-2 budget -- incoherent fp8 noise does not
shrink in a random-walk contraction).

Schedule notes:
  P1 runs the t projection only (~14us of DR matmuls) while the 9.5MB
  of inputs stream: m8/wv ride the scalar HWDGE queue, everything else
  (mask, ident, x_qT8, x_T8, x_nat) the sync queue, each tile ONE
  batched 3D-AP descriptor. A dependency-free fp32 warm-up matmul chain
  covers the first-DMA dead window and holds the HAM clock gate at
  full rate.
  P2 is i-major: each query block accumulates AV over all its key
  blocks in per-bank PSUM tile pairs, the PE stream is software-
  pipelined one key-block ahead (scores of stage n+1 are emitted before
  the transposes+AV of stage n so the PE never waits on the scalar
  exp), causal mask adds touch only a 256-wide slice, and each output
  half finalizes with one ScalarE copy scaled by 1/l straight out of
  PSUM, then DMAs out on the sync HWDGE queue.

(Tried and rejected: fp8 AV / fp8 out-projection (error budget, above);
pairwise AllGather K/V dedup across core pairs -- the NRT collective
path costs ~18us per op serialized, exceeding the PE work it saved.)
"""

import sys

import numpy as np

if "/opt/trn_rl_repo" not in sys.path:
    sys.path.insert(0, "/opt/trn_rl_repo")

B = 4
S = 2048
D = 1024
NB = 8  # query blocks of 128 per core
KH = 8  # 128-row tiles along d_in / d_out
NEG = -1.0e9
WSM = 32.0  # host-side scale on M = Wq @ Wk^T (scores come out 32x)
SCALE = float(D) ** -0.5 / WSM  # exp scale absorbs the 32x
_CACHE = {}
LAST_RESULT = None


def _build_nc():
    import contextlib

    import concourse.bacc as bacc
    import concourse.mybir as mybir
    import concourse.tile as tile

    F32 = mybir.dt.float32
    F8 = mybir.dt.float8e4
    BF = mybir.dt.bfloat16
    DR = mybir.MatmulPerfMode.DoubleRow

    nc = bacc.Bacc(None, target_bir_lowering=False)

    # All inputs arrive HOST-TILED: partition dim first and each
    # partition's SBUF content one contiguous DRAM run (>=4KB DMA lines;
    # the natural layouts produced 512-1024B lines that measured ~half of
    # the 358GB/s HBM rate and starved the kernel start). Need-ordered
    # chunks (key-chunks, q-strips) are SEPARATE dram tensors so every
    # DMA is a rank-matched whole/contiguous slice -- slicing a 4D tensor
    # leaves singleton dims in the APs, which cost ~40ns/matmul when the
    # same trick was tried on the SBUF side.
    x_nat = nc.dram_tensor("x_nat", [128, S // 128, D], BF, kind="ExternalInput")
    xT8_c = [
        nc.dram_tensor(f"xT8_{c}", [128, KH, 512], F8, kind="ExternalInput")
        for c in range(4)
    ]
    xq8_s = [
        nc.dram_tensor(f"xq8_{s}", [128, KH, 512], F8, kind="ExternalInput")
        for s in range(2)
    ]
    m8 = nc.dram_tensor("m8", [128, KH, D], F8, kind="ExternalInput")
    wv = nc.dram_tensor("wv", [128, KH, D], BF, kind="ExternalInput")
    mask = nc.dram_tensor("mask", [128, 2, 512], BF, kind="ExternalInput")
    ident_in = nc.dram_tensor("ident", [128, 128], BF, kind="ExternalInput")
    y = nc.dram_tensor("y", [NB * 128, D], F32, kind="ExternalOutput")
    warm_dram = nc.dram_tensor("warm_scratch", [128, 256], BF)  # HAM warm-up

    with tile.TileContext(nc) as tc:
        with contextlib.ExitStack() as ctx:
            persist = ctx.enter_context(tc.tile_pool(name="persist", bufs=1))

            ident = persist.tile([128, 128], BF)
            mask_sb = persist.tile([128, 2, 512], BF)
            # one tile per 512-col strip: engine-write dependencies are
            # tracked whole-tile, so a single t_T tile would gate the first
            # scores on the LAST strip's PSUM->SBUF copies
            t_Ts = [
                persist.tile([128, KH, 512], F8, name="t_T0"),  # [d_lo,d_hi,sq]
                persist.tile([128, KH, 512], F8, name="t_T1"),
            ]
            # per-chunk key tiles keep the scores rhs a clean 3D AP
            xT_tiles = [
                persist.tile([128, KH, 512], F8, name=f"xTk{c}") for c in range(4)
            ]
            x_keep = persist.tile([128, S // 128, D], BF)  # [s_lo, s_hi, d]
            wv_sb = persist.tile([128, KH, D], BF)  # P2 out-projection
            l_acc = persist.tile([128, NB], F32)

            # sps (the phase-2 score PSUM pool) is opened OUTSIDE phase 1 so
            # it owns two banks that phase-1's mmps never touches: PSUM pool
            # bank reuse orders the first scores matmul behind ALL t_T
            # PSUM->SBUF casts otherwise (measured 1.3us gap).
            sps_pool = ctx.enter_context(
                tc.tile_pool(name="sps", bufs=2, space="PSUM")
            )

            # ---------------- Phase 1: t = x_q @ M projection ----------------
            with (
                tc.tile_pool(name="w8pool", bufs=1) as w8pool,
                tc.tile_pool(name="xq8", bufs=2) as xq8_pool,
                tc.tile_pool(name="mmps", bufs=6, space="PSUM") as mmps_pool,
            ):
                # PE warm-up: dependency-free bf16 matmuls on memset data
                # cover the first-DMA dead window (~7.7-11.2us: m8/xq land
                # ~11us) and bring the HAM clock gate to full rate. bf16
                # single-pass (fp32 ran two passes and overshot data-ready
                # by 2us, delaying the projection). A 16-col slice is
                # written out so the chain isn't dead; kept tiny so the
                # PSUM slot's consumers finish right after the last warm
                # matmul.
                # 9 matmuls ~= 4.0us: sustained past the 3.4us HAM window
                # (7 measured 3.17us and the gate never opened -> cold
                # projection) and ending right at first-input-ready ~12.2us
                warm = persist.tile([128, 512], BF)
                nc.vector.memset(warm, 0.0)
                wps = sps_pool.tile([128, 512], F32, tag="s")
                for m in range(8):
                    nc.tensor.matmul(
                        wps, warm[:, :128], warm,
                        start=(m == 0), stop=(m == 7),
                    )
                nc.vector.tensor_copy(out=warm[:, :16], in_=wps[:, :16])
                nc.gpsimd.dma_start(out=warm_dram[:, :16], in_=warm[:, :16])

                # Weight-side DMAs on the scalar (ACT) HWDGE queue; all x
                # DMAs on the sync (SP) queue. Few LARGE descriptors: each
                # DMA trigger costs ~630ns of queue-engine issue time, so
                # batch per-tile (3D APs) -- only the first m8 halves stay
                # split so the t projection can start ASAP.
                # m8 arrives as kh-PAIR slices (256KB each) matching
                # k_outer's kp consumption order: the first matmuls then
                # gate on ~384KB instead of 1MB (early DMA runs at only
                # ~180-250GB/s per queue, so prerequisite size is the
                # kernel-start lever)
                m8_sb = w8pool.tile([128, KH, D], F8, tag="w8")
                for pp in range(4):
                    nc.scalar.dma_start(
                        out=m8_sb[:, 2 * pp : 2 * pp + 2, :],
                        in_=m8[:, 2 * pp : 2 * pp + 2, :],
                    )
                for hh in range(2):
                    nc.scalar.dma_start(
                        out=wv_sb[:, hh * 4 : (hh + 1) * 4, :],
                        in_=wv[:, hh * 4 : (hh + 1) * 4, :],
                    )

                # sync-queue DMAs strictly in first-need order (HBM BW is
                # the startup constraint at ~358GB/s shared across queues):
                # xq strips gate the t projection (~11us), then per-chunk
                # interleave of x_T8 (scores, need ~25.5+2.2j us), mask/
                # ident (first diag/transpose ~25.5us) and x_nat quarters
                # (AV, need ~26.5+2.2j us).
                xq_tiles = []
                for strip in range(2):
                    xTq = xq8_pool.tile([128, KH, 512], F8, tag="xq8")
                    if strip == 0:
                        # kh-pair slices (128KB): the first k_outer burst
                        # waits only on pair 0
                        for pp in range(4):
                            nc.sync.dma_start(
                                out=xTq[:, 2 * pp : 2 * pp + 2, :],
                                in_=xq8_s[0][:, 2 * pp : 2 * pp + 2, :],
                            )
                    else:
                        nc.sync.dma_start(out=xTq, in_=xq8_s[strip][:, :, :])
                    xq_tiles.append(xTq)

                def xT8_chunk(chunk):
                    nc.sync.dma_start(
                        out=xT_tiles[chunk], in_=xT8_c[chunk][:, :, :]
                    )

                def xnat_quarter(q):
                    nc.sync.dma_start(
                        out=x_keep[:, q * 4 : (q + 1) * 4, :],
                        in_=x_nat[:, q * 4 : (q + 1) * 4, :],
                    )

                xT8_chunk(0)
                nc.sync.dma_start(out=mask_sb, in_=mask[:, :, :])
                nc.sync.dma_start(out=ident, in_=ident_in[:, :])
                xnat_quarter(0)
                xT8_chunk(1)
                xnat_quarter(1)
                xT8_chunk(2)
                xnat_quarter(2)
                xT8_chunk(3)
                xnat_quarter(3)

                def t_cast(dst, src, h):
                    # PSUM->SBUF fp8 casts alternate DVE/ScalarE so the
                    # cast train never becomes the critical path when a
                    # PSUM slot is re-used
                    if h % 2 == 0:
                        nc.vector.tensor_copy(out=dst, in_=src)
                    else:
                        nc.scalar.mul(out=dst, in_=src, mul=1.0)

                def t_segment(strip, k_outer=False):
                    xTq = xq_tiles[strip]
                    t_T = t_Ts[strip]
                    if k_outer:
                        # startup: kp-outer over a 6-bank h-group (6 MMs =
                        # 1.3us per kp-pair, matching the ~1.2us pair-DMA
                        # arrival cadence so the PE neither stalls nor
                        # outruns the stream; 6 banks keeps mmps+sps within
                        # PSUM), then a 2-bank tail once all data is in
                        for hg, nh in ((0, 6), (6, 2)):
                            qpss = []
                            for _h in range(nh):
                                qt = mmps_pool.tile([128, 512], F32, tag="mm")
                                qpss.append(qt)
                            for kp in range(KH // 2):
                                for hh in range(nh):
                                    h = hg + hh
                                    nc.tensor.matmul(
                                        qpss[hh],
                                        m8_sb[:, 2 * kp : 2 * kp + 2, h * 128 : (h + 1) * 128],
                                        xTq[:, 2 * kp : 2 * kp + 2, :],
                                        start=(kp == 0),
                                        stop=(kp == KH // 2 - 1),
                                        perf_mode=DR,
                                    )
                            for hh in range(nh):
                                t_cast(t_T[:, hg + hh, :], qpss[hh], hh)
                        return
                    for h in range(KH):
                        qps = mmps_pool.tile([128, 512], F32, tag="mm")
                        for kp in range(KH // 2):
                            nc.tensor.matmul(
                                qps,
                                m8_sb[:, 2 * kp : 2 * kp + 2, h * 128 : (h + 1) * 128],
                                xTq[:, 2 * kp : 2 * kp + 2, :],
                                start=(kp == 0),
                                stop=(kp == KH // 2 - 1),
                                perf_mode=DR,
                            )
                        t_cast(t_T[:, h, :], qps, h)

                t_segment(0, k_outer=True)
                t_segment(1)

            # ---------------- Phase 2: attention ----------------
            # i-major: each query block i accumulates AV over all its key
            # blocks j=0..i//2 in ONE long PSUM group (no SBUF out_acc at
            # all); the finalize is a single fused (avps * 1/l) PSUM->SBUF
            # op. The PE stream is software-pipelined one j ahead: scores
            # for j+1 are emitted before transposes/AV of j, so the PE
            # never waits on the scalar exp except at the very tail.
            with (
                tc.tile_pool(name="esb", bufs=3) as esb_pool,
                tc.tile_pool(name="etsb", bufs=3) as etsb_pool,
                tc.tile_pool(name="lsb", bufs=4) as lsb_pool,
                tc.tile_pool(name="ysb", bufs=2) as ysb_pool,
                tc.tile_pool(name="etps", bufs=2, space="PSUM") as etps_pool,
                tc.tile_pool(name="avps", bufs=2, space="PSUM") as avps_pool,
                tc.tile_pool(name="outps", bufs=2, space="PSUM") as outps_pool,
                tc.tile_pool(name="usb", bufs=2) as usb_pool,
            ):
                def emit_scores(i, j, diag, ncols):
                    t_T = t_Ts[i // 4]
                    icol = (i % 4) * 128
                    sps = sps_pool.tile([128, 512], F32, tag="s")
                    for kp in range(KH // 2):
                        nc.tensor.matmul(
                            sps[:, :ncols],
                            t_T[:, 2 * kp : 2 * kp + 2, icol : icol + 128],
                            xT_tiles[j][:, 2 * kp : 2 * kp + 2, :ncols],
                            start=(kp == 0),
                            stop=(kp == KH // 2 - 1),
                            perf_mode=DR,
                        )
                    if diag:
                        # the causal boundary only touches a 256-wide slice:
                        # even i -> cols [0:256) of m0; odd i -> [256:512)
                        # of m1 (cols [0:256) are always fully visible)
                        lo = 0 if i % 2 == 0 else 256
                        nc.vector.tensor_add(
                            out=sps[:, lo : lo + 256],
                            in0=sps[:, lo : lo + 256],
                            in1=mask_sb[:, i % 2, lo : lo + 256],
                        )
                    # no accum_out: the accumulator read-out instruction
                    # delays the e_sb write-complete signal ~300ns, which
                    # the E-transposes sit on; l is reduced on DVE inside
                    # emit_av instead (after the et copy, off the PE path)
                    e_sb = esb_pool.tile([128, 512], BF, tag="e")
                    nc.scalar.activation(
                        out=e_sb[:, :ncols],
                        in_=sps[:, :ncols],
                        func=mybir.ActivationFunctionType.Exp,
                        scale=SCALE,
                    )
                    return e_sb

                def emit_av(i, j, diag, ncols, e_sb, av, jmax):
                    njj = ncols // 128
                    etp = etps_pool.tile([128, 1024], BF, tag="et")
                    for jj in range(njj):
                        nc.tensor.transpose(
                            etp[:, jj * 128 : (jj + 1) * 128],
                            e_sb[:, jj * 128 : (jj + 1) * 128],
                            ident,
                        )
                    et = etsb_pool.tile([128, 512], BF, tag="ets")
                    nc.vector.tensor_copy(out=et[:, :ncols], in_=etp[:, :ncols])
                    # l partial: DVE pass over e_sb with a free-axis
                    # accumulator (emitted after the et copy so the AV
                    # stationary load is never queued behind it)
                    lpart = lsb_pool.tile([128, 1], F32, tag="l")
                    lscr = etsb_pool.tile([128, 512], BF, tag="lscr")
                    nc.vector.tensor_scalar(
                        out=lscr[:, :ncols],
                        in0=e_sb[:, :ncols],
                        scalar1=0.0,
                        scalar2=0.0,
                        op0=mybir.AluOpType.add,
                        op1=mybir.AluOpType.add,
                        accum_out=lpart,
                    )
                    if j == 0:
                        nc.vector.tensor_copy(out=l_acc[:, i : i + 1], in_=lpart)
                    else:
                        nc.vector.tensor_add(
                            out=l_acc[:, i : i + 1],
                            in0=l_acc[:, i : i + 1],
                            in1=lpart,
                        )
                    # dh-major with separate per-bank PSUM tiles: each half
                    # finishes accumulating independently so the finalize
                    # of half 0 overlaps the PE work on half 1
                    for dh in range(2):
                        for jj in range(njj):
                            nc.tensor.matmul(
                                av[dh],
                                et[:, jj * 128 : (jj + 1) * 128],
                                x_keep[:, 4 * j + jj, dh * 512 : (dh + 1) * 512],
                                start=(j == 0 and jj == 0),
                                stop=(j == jmax and jj == njj - 1),
                                skip_group_check=True,
                            )

                def finalize_i(i, av, rinv):
                    # U = E@X accumulated in PSUM; out = (U @ Wv) / l -- the
                    # 1/l scale rides the final PSUM->SBUF copy, so the
                    # PE transposes start the moment AV stops (no wait on
                    # the diag exp / reciprocal chain). The U copies split
                    # across ScalarE + DVE to halve that latency.
                    ustage = usb_pool.tile([128, D], BF, tag="u")
                    nc.scalar.mul(out=ustage[:, 0:512], in_=av[0], mul=1.0)
                    nc.vector.tensor_copy(out=ustage[:, 512:1024], in_=av[1])
                    utp = etps_pool.tile([128, 1024], BF, tag="et")
                    for b in range(KH):
                        nc.tensor.transpose(
                            utp[:, b * 128 : (b + 1) * 128],
                            ustage[:, b * 128 : (b + 1) * 128],
                            ident,
                        )
                    ut = usb_pool.tile([128, D], BF, tag="ut")
                    nc.scalar.mul(out=ut[:, 0:512], in_=utp[:, 0:512], mul=1.0)
                    nc.vector.tensor_copy(out=ut[:, 512:D], in_=utp[:, 512:D])
                    ystage = ysb_pool.tile([128, D], F32, tag="y")
                    for dh in range(2):
                        ops = outps_pool.tile([128, 512], F32, tag="o")
                        for kd in range(KH):
                            nc.tensor.matmul(
                                ops,
                                ut[:, kd * 128 : (kd + 1) * 128],
                                wv_sb[:, kd, dh * 512 : (dh + 1) * 512],
                                start=(kd == 0),
                                stop=(kd == KH - 1),
                            )
                        nc.scalar.mul(
                            out=ystage[:, dh * 512 : (dh + 1) * 512],
                            in_=ops,
                            mul=rinv,
                        )
                        # sync HWDGE: faster end-of-kernel drain than SWDGE
                        nc.sync.dma_start(
                            out=y[i * 128 : (i + 1) * 128, dh * 512 : (dh + 1) * 512],
                            in_=ystage[:, dh * 512 : (dh + 1) * 512],
                        )

                # global one-ahead pipeline ACROSS i boundaries: the scores
                # of the next (i, j) stage are always emitted before the
                # transposes+AV of the previous stage, so the PE never
                # waits on the scalar exp -- even through single-j blocks
                # 2 first: its opening stage is non-diag, so the first
                # exp chain has no mask-add latency to cover; 6 last:
                # shortest finalize tail
                order = (2, 0, 1, 3, 4, 5, 7, 6)
                stages = [
                    (i, j, i // 2) for i in order for j in range(i // 2 + 1)
                ]
                avs = {}
                rinvs = {}
                pend = None  # (i, j, diag, ncols, e_sb, jmax) awaiting AV
                for i, j, jmax in stages:
                    if j == 0:
                        av0 = avps_pool.tile([128, 512], F32, tag="av")
                        av1 = avps_pool.tile([128, 512], F32, tag="av")
                        avs[i] = [av0, av1]
                    diag = j == jmax
                    ncols = 256 if (diag and i % 2 == 0) else 512
                    e_sb = emit_scores(i, j, diag, ncols)
                    if pend is not None:
                        pi, pj, pdiag, pncols, pe_sb, pjmax = pend
                        emit_av(pi, pj, pdiag, pncols, pe_sb, avs[pi], pjmax)
                        if pj == pjmax:
                            # l complete once emit_av's DVE reduce lands;
                            # 1/l runs while the PE does the out-projection
                            rinv = lsb_pool.tile([128, 1], F32, tag="r")
                            nc.vector.reciprocal(
                                out=rinv, in_=l_acc[:, pi : pi + 1]
                            )
                            rinvs[pi] = rinv
                            finalize_i(pi, avs[pi], rinvs[pi])
                    pend = (i, j, diag, ncols, e_sb, jmax)
                pi, pj, pdiag, pncols, pe_sb, pjmax = pend
                emit_av(pi, pj, pdiag, pncols, pe_sb, avs[pi], pjmax)
                rinv = lsb_pool.tile([128, 1], F32, tag="r")
                nc.vector.reciprocal(out=rinv, in_=l_acc[:, pi : pi + 1])
                rinvs[pi] = rinv
                finalize_i(pi, avs[pi], rinvs[pi])

    return nc


def _get_nc(finalize=True):
    key = "nc_fin" if finalize else "nc_raw"
    if key not in _CACHE:
        nc = _build_nc()
        if finalize:
            nc.finalize()
        _CACHE[key] = nc
    return _CACHE[key]


def make_in_maps(x, Wq, Wk, Wv):
    """All tensors are host-tiled so every SBUF partition's content is one
    contiguous DRAM run (2-8KB DMA lines; natural layouts gave 512-1024B
    lines at ~half HBM rate), with free-dim chunk granularity outermost."""
    import ml_dtypes

    f8 = ml_dtypes.float8_e4m3
    bf = ml_dtypes.bfloat16
    ident = np.eye(128, dtype=np.float32).astype(bf)
    p = np.arange(128)[:, None]
    c = np.arange(512)[None, :]
    M = np.asarray(Wq, dtype=np.float64) @ np.asarray(Wk, dtype=np.float64).T
    m8_flat = np.ascontiguousarray(M * WSM).astype(np.float32).astype(f8)
    # [d_in, d_out] -> [p=d_in%128, kh=d_in//128, d_out]
    m8_np = np.ascontiguousarray(m8_flat.reshape(KH, 128, D).transpose(1, 0, 2))
    wv_flat = np.ascontiguousarray(Wv, dtype=np.float32).astype(bf)
    wv_np = np.ascontiguousarray(wv_flat.reshape(KH, 128, D).transpose(1, 0, 2))
    in_maps = []
    for core in range(8):
        b, par = core // 2, core % 2
        # mask[0]: boundary block for even local i; mask[1]: odd local i
        m0 = np.where(c <= p + par * 128, 0.0, NEG).astype(bf)
        m1 = np.where(c <= 256 + par * 128 + p, 0.0, NEG).astype(bf)
        mask_np = np.ascontiguousarray(np.stack([m0, m1]).transpose(1, 0, 2))
        xb = np.asarray(x[b], dtype=np.float32)
        xb8 = xb.astype(f8)
        xq8 = xb8.reshape(16, 128, D)[par::2].reshape(NB * 128, D)
        # x^T [d, s] -> per key-chunk c: [p=d%128, kh=d//128, s%512]
        xT8_t = xb8.T.reshape(KH, 128, 4, 512).transpose(2, 1, 0, 3)
        # x_q^T [d, q] -> per strip s: [p, kh, q%512]
        xq8_t = xq8.T.reshape(KH, 128, 2, 512).transpose(2, 1, 0, 3)
        # x [s, d] -> [p=s%128, s_hi=s//128, d]
        xnat_np = np.ascontiguousarray(
            xb.astype(bf).reshape(16, 128, D).transpose(1, 0, 2)
        )
        im = {
            "x_nat": xnat_np,
            "m8": m8_np,
            "wv": wv_np,
            "mask": mask_np,
            "ident": ident,
        }
        for cc in range(4):
            im[f"xT8_{cc}"] = np.ascontiguousarray(xT8_t[cc])
        for ss in range(2):
            im[f"xq8_{ss}"] = np.ascontiguousarray(xq8_t[ss])
        in_maps.append(im)
    return in_maps


def assemble_out(results):
    out = np.empty((B, S, D), dtype=np.float32)
    o4 = out.reshape(B, 16, 128, D)
    for core in range(8):
        b, par = core // 2, core % 2
        o4[b, par::2] = results[core]["y"].reshape(NB, 128, D)
    return out


def _ensure_axon_hooks_shim():
    """bass_utils imports antenv.axon_hooks when BASS_TRACE is set; provide a
    no-op fallback so a stray BASS_TRACE env var can't crash the run."""
    try:
        import antenv.axon_hooks  # noqa: F401
    except ImportError:
        import types

        import antenv

        mod = types.ModuleType("antenv.axon_hooks")
        mod.get_axon_ntff_profile_hook = lambda: None
        mod.set_axon_ntff_profile_hook = lambda h: None
        sys.modules["antenv.axon_hooks"] = mod
        antenv.axon_hooks = mod


def kernel(x, Wq, Wk, Wv):
    global LAST_RESULT
    from concourse.bass_utils import run_bass_kernel_spmd

    _ensure_axon_hooks_shim()
    nc = _get_nc(finalize=True)
    in_maps = make_in_maps(x, Wq, Wk, Wv)
    res = run_bass_kernel_spmd(nc, in_maps, core_ids=list(range(8)))
    LAST_RESULT = res
    return assemble_out(res.results)


# revision 49
# speedup vs baseline: 1.0070x; 1.0070x over previous
"""Causal single-head attention (B=4, S=2048, d=1024) on 8 trn2 NeuronCores.

Sharding: core c -> batch c//2, query-parity c%2. Queries of one batch are
split by even/odd 128-row blocks (interleaved so causal work balances);
every core runs the IDENTICAL program -- the host gathers each core's query
rows into a dense x_qT input, and two per-core [128,512] additive masks
encode the causal boundary.

Score path (weight-fused): scores = (x Wq)(x Wk)^T = x (Wq Wk^T) x^T.
The host folds the two projection weights into M = Wq Wk^T once
(input-independent weight preprocessing), so the kernel computes
t = x_q @ M on its query rows only and contracts t directly against the
fp8 x^T already resident in SBUF -- the K projection disappears
entirely (it was the largest PE block and fully duplicated across each
core pair). Host passes fp8(32*M); the 32x score scale is folded into
the exp. Validated vs f64 reference: rel-max ~1.0e-2 (better than the
q8/k8 route's 1.34e-2 -- one fewer weight-quantization + requant stage).

Mixed precision: t projection + t.x^T scores run as fp8 DoubleRow
matmuls (K=256/instr, 2x PE rate). E (exp output), E^T transposes, AV
and the out-projection run in bf16 (fp8 AV/out-proj tested numerically:
3-4e-2 rel-max, over the 2e-2 budget -- incoherent fp8 noise does not
shrink in a random-walk contraction).

Schedule notes:
  All inputs are host-tiled partition-major so every SBUF partition's
  content is one contiguous DRAM run (>=4KB DMA lines; natural layouts
  gave 512-1024B lines at ~half the HBM rate), and all DMAs are issued
  strictly in first-need order: m8 kh-pairs + wv halves on the scalar
  HWDGE queue; xq0 kh-pairs, xq1, then interleaved x_T8 key-chunks /
  mask / x_nat quarters on the sync queue. A dependency-free bf16
  warm-up matmul chain (~3.6us, past the 3.4us HAM activity window)
  holds the PE clock gate at full rate until the first m8/xq pairs
  land; the t projection then consumes kp-pairs in a 6-PSUM-bank
  kp-outer group sized to the pair-DMA arrival cadence.
  P2 is i-major: each query block accumulates AV over all its key
  blocks in per-bank PSUM tile pairs, the PE stream is software-
  pipelined one key-block ahead (scores of stage n+1 are emitted before
  the transposes+AV of stage n so the PE never waits on the scalar
  exp), causal mask adds touch only a 256-wide slice, and 1/l rides the
  final out-projection PSUM->SBUF copy so nothing downstream of AV
  waits on the diag exp / reciprocal. The score PSUM pool is hoisted
  out of phase 1 so its banks are never re-used from the projection
  pool (PSUM bank re-use orders the first scores behind every t_T
  cast). First phase-2 block is non-diagonal (no mask latency to
  cover); i=6 last (shortest finalize tail).

(Tried and rejected: fp8 AV / fp8 out-projection (error budget, above);
pairwise AllGather K/V dedup across core pairs -- the NRT collective
path costs ~18us per op serialized, exceeding the PE work it saved;
l-reduction on DVE/Pool instead of the exp accumulator -- DVE queuing
delayed the et copies ~2.4us, Pool lacks the accumulator op.)
"""

import sys

import numpy as np

if "/opt/trn_rl_repo" not in sys.path:
    sys.path.insert(0, "/opt/trn_rl_repo")

B = 4
S = 2048
D = 1024
NB = 8  # query blocks of 128 per core
KH = 8  # 128-row tiles along d_in / d_out
NEG = -1.0e9
WSM = 32.0  # host-side scale on M = Wq @ Wk^T (scores come out 32x)
SCALE = float(D) ** -0.5 / WSM  # exp scale absorbs the 32x
_CACHE = {}
LAST_RESULT = None


def _build_nc():
    import contextlib

    import concourse.bacc as bacc
    import concourse.mybir as mybir
    import concourse.tile as tile

    F32 = mybir.dt.float32
    F8 = mybir.dt.float8e4
    BF = mybir.dt.bfloat16
    DR = mybir.MatmulPerfMode.DoubleRow

    nc = bacc.Bacc(None, target_bir_lowering=False)

    # All inputs arrive HOST-TILED: partition dim first and each
    # partition's SBUF content one contiguous DRAM run (>=4KB DMA lines;
    # the natural layouts produced 512-1024B lines that measured ~half of
    # the 358GB/s HBM rate and starved the kernel start). Need-ordered
    # chunks (key-chunks, q-strips) are SEPARATE dram tensors so every
    # DMA is a rank-matched whole/contiguous slice -- slicing a 4D tensor
    # leaves singleton dims in the APs, which cost ~40ns/matmul when the
    # same trick was tried on the SBUF side.
    x_nat = nc.dram_tensor("x_nat", [128, S // 128, D], BF, kind="ExternalInput")
    xT8_c = [
        nc.dram_tensor(f"xT8_{c}", [128, KH, 512], F8, kind="ExternalInput")
        for c in range(4)
    ]
    xq8_s = [
        nc.dram_tensor(f"xq8_{s}", [128, KH, 512], F8, kind="ExternalInput")
        for s in range(2)
    ]
    m8 = nc.dram_tensor("m8", [128, KH, D], F8, kind="ExternalInput")
    wv = nc.dram_tensor("wv", [128, KH, D], BF, kind="ExternalInput")
    mask = nc.dram_tensor("mask", [128, 2, 512], BF, kind="ExternalInput")
    ident_in = nc.dram_tensor("ident", [128, 128], BF, kind="ExternalInput")
    y = nc.dram_tensor("y", [NB * 128, D], F32, kind="ExternalOutput")
    warm_dram = nc.dram_tensor("warm_scratch", [128, 256], BF)  # HAM warm-up

    with tile.TileContext(nc) as tc:
        with contextlib.ExitStack() as ctx:
            persist = ctx.enter_context(tc.tile_pool(name="persist", bufs=1))

            ident = persist.tile([128, 128], BF)
            mask_sb = persist.tile([128, 2, 512], BF)
            # one tile per 512-col strip: engine-write dependencies are
            # tracked whole-tile, so a single t_T tile would gate the first
            # scores on the LAST strip's PSUM->SBUF copies
            t_Ts = [
                persist.tile([128, KH, 512], F8, name="t_T0"),  # [d_lo,d_hi,sq]
                persist.tile([128, KH, 512], F8, name="t_T1"),
            ]
            # per-chunk key tiles keep the scores rhs a clean 3D AP
            xT_tiles = [
                persist.tile([128, KH, 512], F8, name=f"xTk{c}") for c in range(4)
            ]
            x_keep = persist.tile([128, S // 128, D], BF)  # [s_lo, s_hi, d]
            wv_sb = persist.tile([128, KH, D], BF)  # P2 out-projection
            l_acc = persist.tile([128, NB], F32)

            # sps (the phase-2 score PSUM pool) is opened OUTSIDE phase 1 so
            # it owns two banks that phase-1's mmps never touches: PSUM pool
            # bank reuse orders the first scores matmul behind ALL t_T
            # PSUM->SBUF casts otherwise (measured 1.3us gap).
            sps_pool = ctx.enter_context(
                tc.tile_pool(name="sps", bufs=2, space="PSUM")
            )

            # ---------------- Phase 1: t = x_q @ M projection ----------------
            with (
                tc.tile_pool(name="w8pool", bufs=1) as w8pool,
                tc.tile_pool(name="xq8", bufs=2) as xq8_pool,
                tc.tile_pool(name="mmps", bufs=6, space="PSUM") as mmps_pool,
            ):
                # PE warm-up: dependency-free bf16 matmuls on memset data
                # cover the first-DMA dead window (~7.7-11.2us: m8/xq land
                # ~11us) and bring the HAM clock gate to full rate. bf16
                # single-pass (fp32 ran two passes and overshot data-ready
                # by 2us, delaying the projection). A 16-col slice is
                # written out so the chain isn't dead; kept tiny so the
                # PSUM slot's consumers finish right after the last warm
                # matmul.
                # 9 matmuls ~= 4.0us: sustained past the 3.4us HAM window
                # (7 measured 3.17us and the gate never opened -> cold
                # projection) and ending right at first-input-ready ~12.2us
                warm = persist.tile([128, 512], BF)
                nc.vector.memset(warm, 0.0)
                wps = sps_pool.tile([128, 512], F32, tag="s")
                for m in range(8):
                    nc.tensor.matmul(
                        wps, warm[:, :128], warm,
                        start=(m == 0), stop=(m == 7),
                    )
                nc.vector.tensor_copy(out=warm[:, :16], in_=wps[:, :16])
                nc.gpsimd.dma_start(out=warm_dram[:, :16], in_=warm[:, :16])

                # Weight-side DMAs on the scalar (ACT) HWDGE queue; all x
                # DMAs on the sync (SP) queue. Few LARGE descriptors: each
                # DMA trigger costs ~630ns of queue-engine issue time, so
                # batch per-tile (3D APs) -- only the first m8 halves stay
                # split so the t projection can start ASAP.
                # m8 arrives as kh-PAIR slices (256KB each) matching
                # k_outer's kp consumption order: the first matmuls then
                # gate on ~384KB instead of 1MB (early DMA runs at only
                # ~180-250GB/s per queue, so prerequisite size is the
                # kernel-start lever)
                m8_sb = w8pool.tile([128, KH, D], F8, tag="w8")
                for pp in range(4):
                    nc.scalar.dma_start(
                        out=m8_sb[:, 2 * pp : 2 * pp + 2, :],
                        in_=m8[:, 2 * pp : 2 * pp + 2, :],
                    )
                for hh in range(2):
                    nc.scalar.dma_start(
                        out=wv_sb[:, hh * 4 : (hh + 1) * 4, :],
                        in_=wv[:, hh * 4 : (hh + 1) * 4, :],
                    )

                # sync-queue DMAs strictly in first-need order (HBM BW is
                # the startup constraint at ~358GB/s shared across queues):
                # xq strips gate the t projection (~11us), then per-chunk
                # interleave of x_T8 (scores, need ~25.5+2.2j us), mask/
                # ident (first diag/transpose ~25.5us) and x_nat quarters
                # (AV, need ~26.5+2.2j us).
                xq_tiles = []
                for strip in range(2):
                    xTq = xq8_pool.tile([128, KH, 512], F8, tag="xq8")
                    if strip == 0:
                        # kh-pair slices (128KB): the first k_outer burst
                        # waits only on pair 0
                        for pp in range(4):
                            nc.sync.dma_start(
                                out=xTq[:, 2 * pp : 2 * pp + 2, :],
                                in_=xq8_s[0][:, 2 * pp : 2 * pp + 2, :],
                            )
                    else:
                        nc.sync.dma_start(out=xTq, in_=xq8_s[strip][:, :, :])
                    xq_tiles.append(xTq)

                def xT8_chunk(chunk):
                    nc.sync.dma_start(
                        out=xT_tiles[chunk], in_=xT8_c[chunk][:, :, :]
                    )

                def xnat_quarter(q):
                    nc.sync.dma_start(
                        out=x_keep[:, q * 4 : (q + 1) * 4, :],
                        in_=x_nat[:, q * 4 : (q + 1) * 4, :],
                    )

                xT8_chunk(0)
                nc.sync.dma_start(out=mask_sb, in_=mask[:, :, :])
                nc.sync.dma_start(out=ident, in_=ident_in[:, :])
                xnat_quarter(0)
                xT8_chunk(1)
                xnat_quarter(1)
                xT8_chunk(2)
                xnat_quarter(2)
                xT8_chunk(3)
                xnat_quarter(3)

                def t_cast(dst, src, h):
                    # PSUM->SBUF fp8 casts alternate DVE/ScalarE so the
                    # cast train never becomes the critical path when a
                    # PSUM slot is re-used
                    if h % 2 == 0:
                        nc.vector.tensor_copy(out=dst, in_=src)
                    else:
                        nc.scalar.mul(out=dst, in_=src, mul=1.0)

                def t_segment(strip, k_outer=False):
                    xTq = xq_tiles[strip]
                    t_T = t_Ts[strip]
                    if k_outer:
                        # startup: kp-outer over a 6-bank h-group (6 MMs =
                        # 1.3us per kp-pair, matching the ~1.2us pair-DMA
                        # arrival cadence so the PE neither stalls nor
                        # outruns the stream; 6 banks keeps mmps+sps within
                        # PSUM), then a 2-bank tail once all data is in
                        for hg, nh in ((0, 6), (6, 2)):
                            qpss = []
                            for _h in range(nh):
                                qt = mmps_pool.tile([128, 512], F32, tag="mm")
                                qpss.append(qt)
                            for kp in range(KH // 2):
                                for hh in range(nh):
                                    h = hg + hh
                                    nc.tensor.matmul(
                                        qpss[hh],
                                        m8_sb[:, 2 * kp : 2 * kp + 2, h * 128 : (h + 1) * 128],
                                        xTq[:, 2 * kp : 2 * kp + 2, :],
                                        start=(kp == 0),
                                        stop=(kp == KH // 2 - 1),
                                        perf_mode=DR,
                                    )
                            for hh in range(nh):
                                t_cast(t_T[:, hg + hh, :], qpss[hh], hh)
                        return
                    for h in range(KH):
                        qps = mmps_pool.tile([128, 512], F32, tag="mm")
                        for kp in range(KH // 2):
                            nc.tensor.matmul(
                                qps,
                                m8_sb[:, 2 * kp : 2 * kp + 2, h * 128 : (h + 1) * 128],
                                xTq[:, 2 * kp : 2 * kp + 2, :],
                                start=(kp == 0),
                                stop=(kp == KH // 2 - 1),
                                perf_mode=DR,
                            )
                        t_cast(t_T[:, h, :], qps, h)

                t_segment(0, k_outer=True)
                t_segment(1)

            # ---------------- Phase 2: attention ----------------
            # i-major: each query block i accumulates AV over all its key
            # blocks j=0..i//2 in ONE long PSUM group (no SBUF out_acc at
            # all); the finalize is a single fused (avps * 1/l) PSUM->SBUF
            # op. The PE stream is software-pipelined one j ahead: scores
            # for j+1 are emitted before transposes/AV of j, so the PE
            # never waits on the scalar exp except at the very tail.
            with (
                tc.tile_pool(name="esb", bufs=3) as esb_pool,
                tc.tile_pool(name="etsb", bufs=3) as etsb_pool,
                tc.tile_pool(name="lsb", bufs=4) as lsb_pool,
                tc.tile_pool(name="ysb", bufs=2) as ysb_pool,
                tc.tile_pool(name="etps", bufs=2, space="PSUM") as etps_pool,
                tc.tile_pool(name="avps", bufs=2, space="PSUM") as avps_pool,
                tc.tile_pool(name="outps", bufs=2, space="PSUM") as outps_pool,
                tc.tile_pool(name="usb", bufs=2) as usb_pool,
            ):
                def emit_scores(i, j, diag, ncols):
                    t_T = t_Ts[i // 4]
                    icol = (i % 4) * 128
                    sps = sps_pool.tile([128, 512], F32, tag="s")
                    for kp in range(KH // 2):
                        nc.tensor.matmul(
                            sps[:, :ncols],
                            t_T[:, 2 * kp : 2 * kp + 2, icol : icol + 128],
                            xT_tiles[j][:, 2 * kp : 2 * kp + 2, :ncols],
                            start=(kp == 0),
                            stop=(kp == KH // 2 - 1),
                            perf_mode=DR,
                        )
                    if diag:
                        # the causal boundary only touches a 256-wide slice:
                        # even i -> cols [0:256) of m0; odd i -> [256:512)
                        # of m1 (cols [0:256) are always fully visible)
                        lo = 0 if i % 2 == 0 else 256
                        nc.vector.tensor_add(
                            out=sps[:, lo : lo + 256],
                            in0=sps[:, lo : lo + 256],
                            in1=mask_sb[:, i % 2, lo : lo + 256],
                        )
                    # l rides the exp's free-axis accumulator: the extra
                    # ACTIVATION_READ_ACCUMULATOR (~280ns) costs less than
                    # any alternative engine for the reduction (DVE pass
                    # delays the et copies ~2.4us across the kernel; Pool
                    # lacks the accumulator op)
                    e_sb = esb_pool.tile([128, 512], BF, tag="e")
                    lpart = lsb_pool.tile([128, 1], F32, tag="l")
                    nc.scalar.activation(
                        out=e_sb[:, :ncols],
                        in_=sps[:, :ncols],
                        func=mybir.ActivationFunctionType.Exp,
                        scale=SCALE,
                        accum_out=lpart,
                    )
                    if j == 0:
                        nc.vector.tensor_copy(out=l_acc[:, i : i + 1], in_=lpart)
                    else:
                        nc.vector.tensor_add(
                            out=l_acc[:, i : i + 1],
                            in0=l_acc[:, i : i + 1],
                            in1=lpart,
                        )
                    return e_sb

                def emit_av(i, j, diag, ncols, e_sb, av, jmax):
                    njj = ncols // 128
                    etp = etps_pool.tile([128, 1024], BF, tag="et")
                    for jj in range(njj):
                        nc.tensor.transpose(
                            etp[:, jj * 128 : (jj + 1) * 128],
                            e_sb[:, jj * 128 : (jj + 1) * 128],
                            ident,
                        )
                    et = etsb_pool.tile([128, 512], BF, tag="ets")
                    nc.vector.tensor_copy(out=et[:, :ncols], in_=etp[:, :ncols])
                    # dh-major with separate per-bank PSUM tiles: each half
                    # finishes accumulating independently so the finalize
                    # of half 0 overlaps the PE work on half 1
                    for dh in range(2):
                        for jj in range(njj):
                            nc.tensor.matmul(
                                av[dh],
                                et[:, jj * 128 : (jj + 1) * 128],
                                x_keep[:, 4 * j + jj, dh * 512 : (dh + 1) * 512],
                                start=(j == 0 and jj == 0),
                                stop=(j == jmax and jj == njj - 1),
                                skip_group_check=True,
                            )

                def finalize_i(i, av, rinv):
                    # U = E@X accumulated in PSUM; out = (U @ Wv) / l -- the
                    # 1/l scale rides the final PSUM->SBUF copy, so the
                    # PE transposes start the moment AV stops (no wait on
                    # the diag exp / reciprocal chain). The U copies split
                    # across ScalarE + DVE to halve that latency.
                    ustage = usb_pool.tile([128, D], BF, tag="u")
                    nc.scalar.mul(out=ustage[:, 0:512], in_=av[0], mul=1.0)
                    nc.vector.tensor_copy(out=ustage[:, 512:1024], in_=av[1])
                    utp = etps_pool.tile([128, 1024], BF, tag="et")
                    for b in range(KH):
                        nc.tensor.transpose(
                            utp[:, b * 128 : (b + 1) * 128],
                            ustage[:, b * 128 : (b + 1) * 128],
                            ident,
                        )
                    ut = usb_pool.tile([128, D], BF, tag="ut")
                    nc.scalar.mul(out=ut[:, 0:512], in_=utp[:, 0:512], mul=1.0)
                    nc.vector.tensor_copy(out=ut[:, 512:D], in_=utp[:, 512:D])
                    ystage = ysb_pool.tile([128, D], F32, tag="y")
                    for dh in range(2):
                        ops = outps_pool.tile([128, 512], F32, tag="o")
                        for kd in range(KH):
                            nc.tensor.matmul(
                                ops,
                                ut[:, kd * 128 : (kd + 1) * 128],
                                wv_sb[:, kd, dh * 512 : (dh + 1) * 512],
                                start=(kd == 0),
                                stop=(kd == KH - 1),
                            )
                        nc.scalar.mul(
                            out=ystage[:, dh * 512 : (dh + 1) * 512],
                            in_=ops,
                            mul=rinv,
                        )
                        # sync HWDGE: faster end-of-kernel drain than SWDGE
                        nc.sync.dma_start(
                            out=y[i * 128 : (i + 1) * 128, dh * 512 : (dh + 1) * 512],
                            in_=ystage[:, dh * 512 : (dh + 1) * 512],
                        )

                # global one-ahead pipeline ACROSS i boundaries: the scores
                # of the next (i, j) stage are always emitted before the
                # transposes+AV of the previous stage, so the PE never
                # waits on the scalar exp -- even through single-j blocks
                # 2 first: its opening stage is non-diag, so the first
                # exp chain has no mask-add latency to cover; 6 last:
                # shortest finalize tail
                order = (2, 0, 1, 3, 4, 5, 7, 6)
                stages = [
                    (i, j, i // 2) for i in order for j in range(i // 2 + 1)
                ]
                avs = {}
                rinvs = {}

                def process(p):
                    pi, pj, pdiag, pncols, pe_sb, pjmax = p
                    emit_av(pi, pj, pdiag, pncols, pe_sb, avs[pi], pjmax)
                    if pj == pjmax:
                        # l complete once the diag lpart lands; 1/l runs
                        # while the PE does transposes + out-projection
                        rinv = lsb_pool.tile([128, 1], F32, tag="r")
                        nc.vector.reciprocal(out=rinv, in_=l_acc[:, pi : pi + 1])
                        rinvs[pi] = rinv
                        finalize_i(pi, avs[pi], rinvs[pi])

                # depth-2: scores run TWO stages ahead of transposes+AV, so
                # every exp chain has ~1.7us of PE cover (one stage left
                # the first exp + the diag chains ~0.5-1.9us exposed and
                # occasionally idled the PE into a HAM re-throttle). sps
                # bufs=2 still suffices: exp(n) consumes its score PSUM
                # during scores(n+1), before scores(n+2) reuses the slot.
                pq = []
                for i, j, jmax in stages:
                    if j == 0:
                        av0 = avps_pool.tile([128, 512], F32, tag="av")
                        av1 = avps_pool.tile([128, 512], F32, tag="av")
                        avs[i] = [av0, av1]
                    diag = j == jmax
                    ncols = 256 if (diag and i % 2 == 0) else 512
                    e_sb = emit_scores(i, j, diag, ncols)
                    pq.append((i, j, diag, ncols, e_sb, jmax))
                    if len(pq) > 2:
                        process(pq.pop(0))
                for p in pq:
                    process(p)

    return nc


def _get_nc(finalize=True):
    key = "nc_fin" if finalize else "nc_raw"
    if key not in _CACHE:
        nc = _build_nc()
        if finalize:
            nc.finalize()
        _CACHE[key] = nc
    return _CACHE[key]


def make_in_maps(x, Wq, Wk, Wv):
    """All tensors are host-tiled so every SBUF partition's content is one
    contiguous DRAM run (2-8KB DMA lines; natural layouts gave 512-1024B
    lines at ~half HBM rate), with free-dim chunk granularity outermost."""
    import ml_dtypes

    f8 = ml_dtypes.float8_e4m3
    bf = ml_dtypes.bfloat16
    ident = np.eye(128, dtype=np.float32).astype(bf)
    p = np.arange(128)[:, None]
    c = np.arange(512)[None, :]
    M = np.asarray(Wq, dtype=np.float64) @ np.asarray(Wk, dtype=np.float64).T
    m8_flat = np.ascontiguousarray(M * WSM).astype(np.float32).astype(f8)
    # [d_in, d_out] -> [p=d_in%128, kh=d_in//128, d_out]
    m8_np = np.ascontiguousarray(m8_flat.reshape(KH, 128, D).transpose(1, 0, 2))
    wv_flat = np.ascontiguousarray(Wv, dtype=np.float32).astype(bf)
    wv_np = np.ascontiguousarray(wv_flat.reshape(KH, 128, D).transpose(1, 0, 2))
    in_maps = []
    for core in range(8):
        b, par = core // 2, core % 2
        # mask[0]: boundary block for even local i; mask[1]: odd local i
        m0 = np.where(c <= p + par * 128, 0.0, NEG).astype(bf)
        m1 = np.where(c <= 256 + par * 128 + p, 0.0, NEG).astype(bf)
        mask_np = np.ascontiguousarray(np.stack([m0, m1]).transpose(1, 0, 2))
        xb = np.asarray(x[b], dtype=np.float32)
        xb8 = xb.astype(f8)
        xq8 = xb8.reshape(16, 128, D)[par::2].reshape(NB * 128, D)
        # x^T [d, s] -> per key-chunk c: [p=d%128, kh=d//128, s%512]
        xT8_t = xb8.T.reshape(KH, 128, 4, 512).transpose(2, 1, 0, 3)
        # x_q^T [d, q] -> per strip s: [p, kh, q%512]
        xq8_t = xq8.T.reshape(KH, 128, 2, 512).transpose(2, 1, 0, 3)
        # x [s, d] -> [p=s%128, s_hi=s//128, d]
        xnat_np = np.ascontiguousarray(
            xb.astype(bf).reshape(16, 128, D).transpose(1, 0, 2)
        )
        im = {
            "x_nat": xnat_np,
            "m8": m8_np,
            "wv": wv_np,
            "mask": mask_np,
            "ident": ident,
        }
        for cc in range(4):
            im[f"xT8_{cc}"] = np.ascontiguousarray(xT8_t[cc])
        for ss in range(2):
            im[f"xq8_{ss}"] = np.ascontiguousarray(xq8_t[ss])
        in_maps.append(im)
    return in_maps


def assemble_out(results):
    out = np.empty((B, S, D), dtype=np.float32)
    o4 = out.reshape(B, 16, 128, D)
    for core in range(8):
        b, par = core // 2, core % 2
        o4[b, par::2] = results[core]["y"].reshape(NB, 128, D)
    return out


def _ensure_axon_hooks_shim():
    """bass_utils imports antenv.axon_hooks when BASS_TRACE is set; provide a
    no-op fallback so a stray BASS_TRACE env var can't crash the run."""
    try:
        import antenv.axon_hooks  # noqa: F401
    except ImportError:
        import types

        import antenv

        mod = types.ModuleType("antenv.axon_hooks")
        mod.get_axon_ntff_profile_hook = lambda: None
        mod.set_axon_ntff_profile_hook = lambda h: None
        sys.modules["antenv.axon_hooks"] = mod
        antenv.axon_hooks = mod


def kernel(x, Wq, Wk, Wv):
    global LAST_RESULT
    from concourse.bass_utils import run_bass_kernel_spmd

    _ensure_axon_hooks_shim()
    nc = _get_nc(finalize=True)
    in_maps = make_in_maps(x, Wq, Wk, Wv)
    res = run_bass_kernel_spmd(nc, in_maps, core_ids=list(range(8)))
    LAST_RESULT = res
    return assemble_out(res.results)


# revision 52
# speedup vs baseline: 1.0088x; 1.0017x over previous
"""Causal single-head attention (B=4, S=2048, d=1024) on 8 trn2 NeuronCores.

Sharding: core c -> batch c//2, query-parity c%2. Queries of one batch are
split by even/odd 128-row blocks (interleaved so causal work balances);
every core runs the IDENTICAL program -- the host gathers each core's query
rows into a dense x_qT input, and two per-core [128,512] additive masks
encode the causal boundary.

Score path (weight-fused): scores = (x Wq)(x Wk)^T = x (Wq Wk^T) x^T.
The host folds the two projection weights into M = Wq Wk^T once
(input-independent weight preprocessing), so the kernel computes
t = x_q @ M on its query rows only and contracts t directly against the
fp8 x^T already resident in SBUF -- the K projection disappears
entirely (it was the largest PE block and fully duplicated across each
core pair). Host passes fp8(32*M); the 32x score scale is folded into
the exp. Validated vs f64 reference: rel-max ~1.0e-2 (better than the
q8/k8 route's 1.34e-2 -- one fewer weight-quantization + requant stage).

Mixed precision: t projection + t.x^T scores run as fp8 DoubleRow
matmuls (K=256/instr, 2x PE rate). E (exp output), E^T transposes, AV
and the out-projection run in bf16 (fp8 AV/out-proj tested numerically:
3-4e-2 rel-max, over the 2e-2 budget -- incoherent fp8 noise does not
shrink in a random-walk contraction).

Schedule notes:
  All inputs are host-tiled partition-major so every SBUF partition's
  content is one contiguous DRAM run (>=4KB DMA lines; natural layouts
  gave 512-1024B lines at ~half the HBM rate), and all DMAs are issued
  strictly in first-need order: m8 kh-pairs + wv halves on the scalar
  HWDGE queue; xq0 kh-pairs, xq1, then interleaved x_T8 key-chunks /
  mask / x_nat quarters on the sync queue. A dependency-free bf16
  warm-up matmul chain (~3.6us, past the 3.4us HAM activity window)
  holds the PE clock gate at full rate until the first m8/xq pairs
  land; the t projection then consumes kp-pairs in a 6-PSUM-bank
  kp-outer group sized to the pair-DMA arrival cadence.
  P2 is i-major: each query block accumulates AV over all its key
  blocks in per-bank PSUM tile pairs, the PE stream is software-
  pipelined one key-block ahead (scores of stage n+1 are emitted before
  the transposes+AV of stage n so the PE never waits on the scalar
  exp), causal mask adds touch only a 256-wide slice, and 1/l rides the
  final out-projection PSUM->SBUF copy so nothing downstream of AV
  waits on the diag exp / reciprocal. The score PSUM pool is hoisted
  out of phase 1 so its banks are never re-used from the projection
  pool (PSUM bank re-use orders the first scores behind every t_T
  cast). First phase-2 block is non-diagonal (no mask latency to
  cover); i=6 last (shortest finalize tail).

(Tried and rejected: fp8 AV / fp8 out-projection (error budget, above);
pairwise AllGather K/V dedup across core pairs -- the NRT collective
path costs ~18us per op serialized, exceeding the PE work it saved;
l-reduction on DVE/Pool instead of the exp accumulator -- DVE queuing
delayed the et copies ~2.4us, Pool lacks the accumulator op.)
"""

import sys

import numpy as np

if "/opt/trn_rl_repo" not in sys.path:
    sys.path.insert(0, "/opt/trn_rl_repo")

B = 4
S = 2048
D = 1024
NB = 8  # query blocks of 128 per core
KH = 8  # 128-row tiles along d_in / d_out
NEG = -1.0e9
WSM = 32.0  # host-side scale on M = Wq @ Wk^T (scores come out 32x)
SCALE = float(D) ** -0.5 / WSM  # exp scale absorbs the 32x
_CACHE = {}
LAST_RESULT = None


def _build_nc():
    import contextlib

    import concourse.bacc as bacc
    import concourse.mybir as mybir
    import concourse.tile as tile

    F32 = mybir.dt.float32
    F8 = mybir.dt.float8e4
    BF = mybir.dt.bfloat16
    DR = mybir.MatmulPerfMode.DoubleRow

    nc = bacc.Bacc(None, target_bir_lowering=False)

    # All inputs arrive HOST-TILED: partition dim first and each
    # partition's SBUF content one contiguous DRAM run (>=4KB DMA lines;
    # the natural layouts produced 512-1024B lines that measured ~half of
    # the 358GB/s HBM rate and starved the kernel start). Need-ordered
    # chunks (key-chunks, q-strips) are SEPARATE dram tensors so every
    # DMA is a rank-matched whole/contiguous slice -- slicing a 4D tensor
    # leaves singleton dims in the APs, which cost ~40ns/matmul when the
    # same trick was tried on the SBUF side.
    x_nat = nc.dram_tensor("x_nat", [128, S // 128, D], BF, kind="ExternalInput")
    xT8_c = [
        nc.dram_tensor(f"xT8_{c}", [128, KH, 512], F8, kind="ExternalInput")
        for c in range(4)
    ]
    xq8_s = [
        nc.dram_tensor(f"xq8_{s}", [128, KH, 512], F8, kind="ExternalInput")
        for s in range(2)
    ]
    m8 = nc.dram_tensor("m8", [128, KH, D], F8, kind="ExternalInput")
    wv = nc.dram_tensor("wv", [128, KH, D], BF, kind="ExternalInput")
    mask = nc.dram_tensor("mask", [128, 2, 512], BF, kind="ExternalInput")
    ident_in = nc.dram_tensor("ident", [128, 128], BF, kind="ExternalInput")
    y = nc.dram_tensor("y", [NB * 128, D], F32, kind="ExternalOutput")
    warm_dram = nc.dram_tensor("warm_scratch", [128, 256], BF)  # HAM warm-up

    with tile.TileContext(nc) as tc:
        with contextlib.ExitStack() as ctx:
            persist = ctx.enter_context(tc.tile_pool(name="persist", bufs=1))

            ident = persist.tile([128, 128], BF)
            mask_sb = persist.tile([128, 2, 512], BF)
            # one tile per 512-col strip: engine-write dependencies are
            # tracked whole-tile, so a single t_T tile would gate the first
            # scores on the LAST strip's PSUM->SBUF copies
            t_Ts = [
                persist.tile([128, KH, 512], F8, name="t_T0"),  # [d_lo,d_hi,sq]
                persist.tile([128, KH, 512], F8, name="t_T1"),
            ]
            # per-chunk key tiles keep the scores rhs a clean 3D AP
            xT_tiles = [
                persist.tile([128, KH, 512], F8, name=f"xTk{c}") for c in range(4)
            ]
            x_keep = persist.tile([128, S // 128, D], BF)  # [s_lo, s_hi, d]
            wv_sb = persist.tile([128, KH, D], BF)  # P2 out-projection
            l_acc = persist.tile([128, NB], F32)

            # sps (the phase-2 score PSUM pool) is opened OUTSIDE phase 1 so
            # it owns two banks that phase-1's mmps never touches: PSUM pool
            # bank reuse orders the first scores matmul behind ALL t_T
            # PSUM->SBUF casts otherwise (measured 1.3us gap).
            sps_pool = ctx.enter_context(
                tc.tile_pool(name="sps", bufs=2, space="PSUM")
            )

            # ---------------- Phase 1: t = x_q @ M projection ----------------
            with (
                tc.tile_pool(name="w8pool", bufs=1) as w8pool,
                tc.tile_pool(name="xq8", bufs=2) as xq8_pool,
                tc.tile_pool(name="mmps", bufs=6, space="PSUM") as mmps_pool,
            ):
                # PE warm-up: dependency-free bf16 matmuls on memset data
                # cover the first-DMA dead window (~7.7-11.2us: m8/xq land
                # ~11us) and bring the HAM clock gate to full rate. bf16
                # single-pass (fp32 ran two passes and overshot data-ready
                # by 2us, delaying the projection). A 16-col slice is
                # written out so the chain isn't dead; kept tiny so the
                # PSUM slot's consumers finish right after the last warm
                # matmul.
                # 9 matmuls ~= 4.0us: sustained past the 3.4us HAM window
                # (7 measured 3.17us and the gate never opened -> cold
                # projection) and ending right at first-input-ready ~12.2us
                warm = persist.tile([128, 512], BF)
                nc.vector.memset(warm, 0.0)
                wps = sps_pool.tile([128, 512], F32, tag="s")
                for m in range(8):
                    nc.tensor.matmul(
                        wps, warm[:, :128], warm,
                        start=(m == 0), stop=(m == 7),
                    )
                nc.vector.tensor_copy(out=warm[:, :16], in_=wps[:, :16])
                nc.gpsimd.dma_start(out=warm_dram[:, :16], in_=warm[:, :16])

                # Weight-side DMAs on the scalar (ACT) HWDGE queue; all x
                # DMAs on the sync (SP) queue. Few LARGE descriptors: each
                # DMA trigger costs ~630ns of queue-engine issue time, so
                # batch per-tile (3D APs) -- only the first m8 halves stay
                # split so the t projection can start ASAP.
                # m8 arrives as kh-PAIR slices (256KB each) matching
                # k_outer's kp consumption order: the first matmuls then
                # gate on ~384KB instead of 1MB (early DMA runs at only
                # ~180-250GB/s per queue, so prerequisite size is the
                # kernel-start lever)
                m8_sb = w8pool.tile([128, KH, D], F8, tag="w8")
                for pp in range(4):
                    nc.scalar.dma_start(
                        out=m8_sb[:, 2 * pp : 2 * pp + 2, :],
                        in_=m8[:, 2 * pp : 2 * pp + 2, :],
                    )
                for hh in range(2):
                    nc.scalar.dma_start(
                        out=wv_sb[:, hh * 4 : (hh + 1) * 4, :],
                        in_=wv[:, hh * 4 : (hh + 1) * 4, :],
                    )

                # sync-queue DMAs strictly in first-need order (HBM BW is
                # the startup constraint at ~358GB/s shared across queues):
                # xq strips gate the t projection (~11us), then per-chunk
                # interleave of x_T8 (scores, need ~25.5+2.2j us), mask/
                # ident (first diag/transpose ~25.5us) and x_nat quarters
                # (AV, need ~26.5+2.2j us).
                xq_tiles = []
                for strip in range(2):
                    xTq = xq8_pool.tile([128, KH, 512], F8, tag="xq8")
                    if strip == 0:
                        # kh-pair slices (128KB): the first k_outer burst
                        # waits only on pair 0
                        for pp in range(4):
                            nc.sync.dma_start(
                                out=xTq[:, 2 * pp : 2 * pp + 2, :],
                                in_=xq8_s[0][:, 2 * pp : 2 * pp + 2, :],
                            )
                    else:
                        nc.sync.dma_start(out=xTq, in_=xq8_s[strip][:, :, :])
                    xq_tiles.append(xTq)

                def xT8_chunk(chunk):
                    nc.sync.dma_start(
                        out=xT_tiles[chunk], in_=xT8_c[chunk][:, :, :]
                    )

                def xnat_quarter(q):
                    nc.sync.dma_start(
                        out=x_keep[:, q * 4 : (q + 1) * 4, :],
                        in_=x_nat[:, q * 4 : (q + 1) * 4, :],
                    )

                xT8_chunk(0)
                nc.sync.dma_start(out=mask_sb, in_=mask[:, :, :])
                nc.sync.dma_start(out=ident, in_=ident_in[:, :])
                xnat_quarter(0)
                xT8_chunk(1)
                xnat_quarter(1)
                xT8_chunk(2)
                xnat_quarter(2)
                xT8_chunk(3)
                xnat_quarter(3)

                def t_cast(dst, src, h):
                    # PSUM->SBUF fp8 casts alternate DVE/ScalarE so the
                    # cast train never becomes the critical path when a
                    # PSUM slot is re-used
                    if h % 2 == 0:
                        nc.vector.tensor_copy(out=dst, in_=src)
                    else:
                        nc.scalar.mul(out=dst, in_=src, mul=1.0)

                def t_segment(strip, k_outer=False):
                    xTq = xq_tiles[strip]
                    t_T = t_Ts[strip]
                    if k_outer:
                        # startup: kp-outer over a 6-bank h-group (6 MMs =
                        # 1.3us per kp-pair, matching the ~1.2us pair-DMA
                        # arrival cadence so the PE neither stalls nor
                        # outruns the stream; 6 banks keeps mmps+sps within
                        # PSUM), then a 2-bank tail once all data is in
                        for hg, nh in ((0, 6), (6, 2)):
                            qpss = []
                            for _h in range(nh):
                                qt = mmps_pool.tile([128, 512], F32, tag="mm")
                                qpss.append(qt)
                            for kp in range(KH // 2):
                                for hh in range(nh):
                                    h = hg + hh
                                    nc.tensor.matmul(
                                        qpss[hh],
                                        m8_sb[:, 2 * kp : 2 * kp + 2, h * 128 : (h + 1) * 128],
                                        xTq[:, 2 * kp : 2 * kp + 2, :],
                                        start=(kp == 0),
                                        stop=(kp == KH // 2 - 1),
                                        perf_mode=DR,
                                    )
                            for hh in range(nh):
                                t_cast(t_T[:, hg + hh, :], qpss[hh], hh)
                        return
                    for h in range(KH):
                        qps = mmps_pool.tile([128, 512], F32, tag="mm")
                        for kp in range(KH // 2):
                            nc.tensor.matmul(
                                qps,
                                m8_sb[:, 2 * kp : 2 * kp + 2, h * 128 : (h + 1) * 128],
                                xTq[:, 2 * kp : 2 * kp + 2, :],
                                start=(kp == 0),
                                stop=(kp == KH // 2 - 1),
                                perf_mode=DR,
                            )
                        t_cast(t_T[:, h, :], qps, h)

                t_segment(0, k_outer=True)
                t_segment(1)

            # ---------------- Phase 2: attention ----------------
            # i-major: each query block i accumulates AV over all its key
            # blocks j=0..i//2 in ONE long PSUM group (no SBUF out_acc at
            # all); the finalize is a single fused (avps * 1/l) PSUM->SBUF
            # op. The PE stream is software-pipelined one j ahead: scores
            # for j+1 are emitted before transposes/AV of j, so the PE
            # never waits on the scalar exp except at the very tail.
            with (
                tc.tile_pool(name="esb", bufs=3) as esb_pool,
                tc.tile_pool(name="etsb", bufs=3) as etsb_pool,
                tc.tile_pool(name="lsb", bufs=4) as lsb_pool,
                tc.tile_pool(name="ysb", bufs=2) as ysb_pool,
                tc.tile_pool(name="etps", bufs=2, space="PSUM") as etps_pool,
                tc.tile_pool(name="avps", bufs=2, space="PSUM") as avps_pool,
                tc.tile_pool(name="outps", bufs=2, space="PSUM") as outps_pool,
                tc.tile_pool(name="usb", bufs=2) as usb_pool,
            ):
                def emit_scores(i, j, diag, ncols):
                    t_T = t_Ts[i // 4]
                    icol = (i % 4) * 128
                    sps = sps_pool.tile([128, 512], F32, tag="s")
                    for kp in range(KH // 2):
                        nc.tensor.matmul(
                            sps[:, :ncols],
                            t_T[:, 2 * kp : 2 * kp + 2, icol : icol + 128],
                            xT_tiles[j][:, 2 * kp : 2 * kp + 2, :ncols],
                            start=(kp == 0),
                            stop=(kp == KH // 2 - 1),
                            perf_mode=DR,
                        )
                    if diag:
                        # the causal boundary only touches a 256-wide slice:
                        # even i -> cols [0:256) of m0; odd i -> [256:512)
                        # of m1 (cols [0:256) are always fully visible)
                        lo = 0 if i % 2 == 0 else 256
                        nc.vector.tensor_add(
                            out=sps[:, lo : lo + 256],
                            in0=sps[:, lo : lo + 256],
                            in1=mask_sb[:, i % 2, lo : lo + 256],
                        )
                    # l rides the exp's free-axis accumulator: the extra
                    # ACTIVATION_READ_ACCUMULATOR (~280ns) costs less than
                    # any alternative engine for the reduction (DVE pass
                    # delays the et copies ~2.4us across the kernel; Pool
                    # lacks the accumulator op)
                    e_sb = esb_pool.tile([128, 512], BF, tag="e")
                    lpart = lsb_pool.tile([128, 1], F32, tag="l")
                    nc.scalar.activation(
                        out=e_sb[:, :ncols],
                        in_=sps[:, :ncols],
                        func=mybir.ActivationFunctionType.Exp,
                        scale=SCALE,
                        accum_out=lpart,
                    )
                    if j == 0:
                        nc.vector.tensor_copy(out=l_acc[:, i : i + 1], in_=lpart)
                    else:
                        nc.vector.tensor_add(
                            out=l_acc[:, i : i + 1],
                            in0=l_acc[:, i : i + 1],
                            in1=lpart,
                        )
                    return e_sb

                def emit_av(i, j, diag, ncols, e_sb, av, jmax):
                    njj = ncols // 128
                    etp = etps_pool.tile([128, 1024], BF, tag="et")
                    for jj in range(njj):
                        nc.tensor.transpose(
                            etp[:, jj * 128 : (jj + 1) * 128],
                            e_sb[:, jj * 128 : (jj + 1) * 128],
                            ident,
                        )
                    et = etsb_pool.tile([128, 512], BF, tag="ets")
                    nc.vector.tensor_copy(out=et[:, :ncols], in_=etp[:, :ncols])
                    # dh-major with separate per-bank PSUM tiles: each half
                    # finishes accumulating independently so the finalize
                    # of half 0 overlaps the PE work on half 1
                    for dh in range(2):
                        for jj in range(njj):
                            nc.tensor.matmul(
                                av[dh],
                                et[:, jj * 128 : (jj + 1) * 128],
                                x_keep[:, 4 * j + jj, dh * 512 : (dh + 1) * 512],
                                start=(j == 0 and jj == 0),
                                stop=(j == jmax and jj == njj - 1),
                                skip_group_check=True,
                            )

                def finalize_a(i, av):
                    # U = E@X accumulated in PSUM: copy out (split across
                    # ScalarE + DVE) and transpose on the PE. The 1/l
                    # scale rides the final PSUM->SBUF copy in finalize_b,
                    # so nothing here waits on the diag exp / reciprocal.
                    ustage = usb_pool.tile([128, D], BF, tag="u")
                    nc.scalar.mul(out=ustage[:, 0:512], in_=av[0], mul=1.0)
                    nc.vector.tensor_copy(out=ustage[:, 512:1024], in_=av[1])
                    utp = etps_pool.tile([128, 1024], BF, tag="et")
                    for b in range(KH):
                        nc.tensor.transpose(
                            utp[:, b * 128 : (b + 1) * 128],
                            ustage[:, b * 128 : (b + 1) * 128],
                            ident,
                        )
                    ut = usb_pool.tile([128, D], BF, tag="ut")
                    nc.scalar.mul(out=ut[:, 0:512], in_=utp[:, 0:512], mul=1.0)
                    nc.vector.tensor_copy(out=ut[:, 512:D], in_=utp[:, 512:D])
                    return ut

                def finalize_b(i, ut, rinv):
                    # out = (U @ Wv) / l. Deferred one pipeline pop after
                    # finalize_a so the next stage's scores+AV cover the
                    # ut PSUM->SBUF copy latency (measured ~0.8us exposed
                    # when the out-projection followed it immediately).
                    ystage = ysb_pool.tile([128, D], F32, tag="y")
                    for dh in range(2):
                        ops = outps_pool.tile([128, 512], F32, tag="o")
                        for kd in range(KH):
                            nc.tensor.matmul(
                                ops,
                                ut[:, kd * 128 : (kd + 1) * 128],
                                wv_sb[:, kd, dh * 512 : (dh + 1) * 512],
                                start=(kd == 0),
                                stop=(kd == KH - 1),
                            )
                        nc.scalar.mul(
                            out=ystage[:, dh * 512 : (dh + 1) * 512],
                            in_=ops,
                            mul=rinv,
                        )
                        # sync HWDGE: faster end-of-kernel drain than SWDGE
                        nc.sync.dma_start(
                            out=y[i * 128 : (i + 1) * 128, dh * 512 : (dh + 1) * 512],
                            in_=ystage[:, dh * 512 : (dh + 1) * 512],
                        )

                # global one-ahead pipeline ACROSS i boundaries: the scores
                # of the next (i, j) stage are always emitted before the
                # transposes+AV of the previous stage, so the PE never
                # waits on the scalar exp -- even through single-j blocks
                # 2 first: its opening stage is non-diag, so the first
                # exp chain has no mask-add latency to cover; 6 last:
                # shortest finalize tail
                order = (2, 0, 1, 3, 4, 5, 7, 6)
                stages = [
                    (i, j, i // 2) for i in order for j in range(i // 2 + 1)
                ]
                avs = {}
                rinvs = {}
                fin_pending = []  # at most one deferred (i, ut, rinv)

                def process(p):
                    pi, pj, pdiag, pncols, pe_sb, pjmax = p
                    emit_av(pi, pj, pdiag, pncols, pe_sb, avs[pi], pjmax)
                    if fin_pending:
                        finalize_b(*fin_pending.pop())
                    if pj == pjmax:
                        # l complete once the diag lpart lands; 1/l runs
                        # while the PE does transposes + out-projection
                        rinv = lsb_pool.tile([128, 1], F32, tag="r")
                        nc.vector.reciprocal(out=rinv, in_=l_acc[:, pi : pi + 1])
                        rinvs[pi] = rinv
                        ut = finalize_a(pi, avs[pi])
                        fin_pending.append((pi, ut, rinv))

                # depth-2: scores run TWO stages ahead of transposes+AV, so
                # every exp chain has ~1.7us of PE cover (one stage left
                # the first exp + the diag chains ~0.5-1.9us exposed and
                # occasionally idled the PE into a HAM re-throttle). sps
                # bufs=2 still suffices: exp(n) consumes its score PSUM
                # during scores(n+1), before scores(n+2) reuses the slot.
                pq = []
                for i, j, jmax in stages:
                    if j == 0:
                        av0 = avps_pool.tile([128, 512], F32, tag="av")
                        av1 = avps_pool.tile([128, 512], F32, tag="av")
                        avs[i] = [av0, av1]
                    diag = j == jmax
                    ncols = 256 if (diag and i % 2 == 0) else 512
                    e_sb = emit_scores(i, j, diag, ncols)
                    pq.append((i, j, diag, ncols, e_sb, jmax))
                    if len(pq) > 2:
                        process(pq.pop(0))
                for p in pq:
                    process(p)
                if fin_pending:
                    finalize_b(*fin_pending.pop())

    return nc


def _get_nc(finalize=True):
    key = "nc_fin" if finalize else "nc_raw"
    if key not in _CACHE:
        nc = _build_nc()
        if finalize:
            nc.finalize()
        _CACHE[key] = nc
    return _CACHE[key]


def make_in_maps(x, Wq, Wk, Wv):
    """All tensors are host-tiled so every SBUF partition's content is one
    contiguous DRAM run (2-8KB DMA lines; natural layouts gave 512-1024B
    lines at ~half HBM rate), with free-dim chunk granularity outermost."""
    import ml_dtypes

    f8 = ml_dtypes.float8_e4m3
    bf = ml_dtypes.bfloat16
    ident = np.eye(128, dtype=np.float32).astype(bf)
    p = np.arange(128)[:, None]
    c = np.arange(512)[None, :]
    M = np.asarray(Wq, dtype=np.float64) @ np.asarray(Wk, dtype=np.float64).T
    m8_flat = np.ascontiguousarray(M * WSM).astype(np.float32).astype(f8)
    # [d_in, d_out] -> [p=d_in%128, kh=d_in//128, d_out]
    m8_np = np.ascontiguousarray(m8_flat.reshape(KH, 128, D).transpose(1, 0, 2))
    wv_flat = np.ascontiguousarray(Wv, dtype=np.float32).astype(bf)
    wv_np = np.ascontiguousarray(wv_flat.reshape(KH, 128, D).transpose(1, 0, 2))
    in_maps = []
    for core in range(8):
        b, par = core // 2, core % 2
        # mask[0]: boundary block for even local i; mask[1]: odd local i
        m0 = np.where(c <= p + par * 128, 0.0, NEG).astype(bf)
        m1 = np.where(c <= 256 + par * 128 + p, 0.0, NEG).astype(bf)
        mask_np = np.ascontiguousarray(np.stack([m0, m1]).transpose(1, 0, 2))
        xb = np.asarray(x[b], dtype=np.float32)
        xb8 = xb.astype(f8)
        xq8 = xb8.reshape(16, 128, D)[par::2].reshape(NB * 128, D)
        # x^T [d, s] -> per key-chunk c: [p=d%128, kh=d//128, s%512]
        xT8_t = xb8.T.reshape(KH, 128, 4, 512).transpose(2, 1, 0, 3)
        # x_q^T [d, q] -> per strip s: [p, kh, q%512]
        xq8_t = xq8.T.reshape(KH, 128, 2, 512).transpose(2, 1, 0, 3)
        # x [s, d] -> [p=s%128, s_hi=s//128, d]
        xnat_np = np.ascontiguousarray(
            xb.astype(bf).reshape(16, 128, D).transpose(1, 0, 2)
        )
        im = {
            "x_nat": xnat_np,
            "m8": m8_np,
            "wv": wv_np,
            "mask": mask_np,
            "ident": ident,
        }
        for cc in range(4):
            im[f"xT8_{cc}"] = np.ascontiguousarray(xT8_t[cc])
        for ss in range(2):
            im[f"xq8_{ss}"] = np.ascontiguousarray(xq8_t[ss])
        in_maps.append(im)
    return in_maps


def assemble_out(results):
    out = np.empty((B, S, D), dtype=np.float32)
    o4 = out.reshape(B, 16, 128, D)
    for core in range(8):
        b, par = core // 2, core % 2
        o4[b, par::2] = results[core]["y"].reshape(NB, 128, D)
    return out


def _ensure_axon_hooks_shim():
    """bass_utils imports antenv.axon_hooks when BASS_TRACE is set; provide a
    no-op fallback so a stray BASS_TRACE env var can't crash the run."""
    try:
        import antenv.axon_hooks  # noqa: F401
    except ImportError:
        import types

        import antenv

        mod = types.ModuleType("antenv.axon_hooks")
        mod.get_axon_ntff_profile_hook = lambda: None
        mod.set_axon_ntff_profile_hook = lambda h: None
        sys.modules["antenv.axon_hooks"] = mod
        antenv.axon_hooks = mod


def kernel(x, Wq, Wk, Wv):
    global LAST_RESULT
    from concourse.bass_utils import run_bass_kernel_spmd

    _ensure_axon_hooks_shim()
    nc = _get_nc(finalize=True)
    in_maps = make_in_maps(x, Wq, Wk, Wv)
    res = run_bass_kernel_spmd(nc, in_maps, core_ids=list(range(8)))
    LAST_RESULT = res
    return assemble_out(res.results)


# revision 54
# speedup vs baseline: 1.0187x; 1.0098x over previous
"""Causal single-head attention (B=4, S=2048, d=1024) on 8 trn2 NeuronCores.

Sharding: core c -> batch c//2, query-parity c%2. Queries of one batch are
split by even/odd 128-row blocks (interleaved so causal work balances);
every core runs the IDENTICAL program -- the host gathers each core's query
rows into a dense x_qT input, and two per-core [128,512] additive masks
encode the causal boundary.

Score path (weight-fused): scores = (x Wq)(x Wk)^T = x (Wq Wk^T) x^T.
The host folds the two projection weights into M = Wq Wk^T once
(input-independent weight preprocessing), so the kernel computes
t = x_q @ M on its query rows only and contracts t directly against the
fp8 x^T already resident in SBUF -- the K projection disappears
entirely (it was the largest PE block and fully duplicated across each
core pair). Host passes fp8(32*M); the 32x score scale is folded into
the exp. Validated vs f64 reference: rel-max ~1.0e-2 (better than the
q8/k8 route's 1.34e-2 -- one fewer weight-quantization + requant stage).

Mixed precision: t projection + t.x^T scores run as fp8 DoubleRow
matmuls (K=256/instr, 2x PE rate). E (exp output), E^T transposes, AV
and the out-projection run in bf16 (fp8 AV/out-proj tested numerically:
3-4e-2 rel-max, over the 2e-2 budget -- incoherent fp8 noise does not
shrink in a random-walk contraction).

Schedule notes:
  All inputs are host-tiled partition-major so every SBUF partition's
  content is one contiguous DRAM run (>=4KB DMA lines; natural layouts
  gave 512-1024B lines at ~half the HBM rate), and all DMAs are issued
  strictly in first-need order: m8 kh-pairs + wv halves on the scalar
  HWDGE queue; xq0 kh-pairs, xq1, then interleaved x_T8 key-chunks /
  mask / x_nat quarters on the sync queue. A dependency-free bf16
  warm-up matmul chain (~3.6us, past the 3.4us HAM activity window)
  holds the PE clock gate at full rate until the first m8/xq pairs
  land; the t projection then consumes kp-pairs in a 6-PSUM-bank
  kp-outer group sized to the pair-DMA arrival cadence.
  P2 is i-major: each query block accumulates AV over all its key
  blocks in per-bank PSUM tile pairs, the PE stream is software-
  pipelined one key-block ahead (scores of stage n+1 are emitted before
  the transposes+AV of stage n so the PE never waits on the scalar
  exp), causal mask adds touch only a 256-wide slice, and 1/l rides the
  final out-projection PSUM->SBUF copy so nothing downstream of AV
  waits on the diag exp / reciprocal. The score PSUM pool is hoisted
  out of phase 1 so its banks are never re-used from the projection
  pool (PSUM bank re-use orders the first scores behind every t_T
  cast). First phase-2 block is non-diagonal (no mask latency to
  cover); i=6 last (shortest finalize tail).

(Tried and rejected: fp8 AV / fp8 out-projection (error budget, above);
pairwise AllGather K/V dedup across core pairs -- the NRT collective
path costs ~18us per op serialized, exceeding the PE work it saved;
l-reduction on DVE/Pool instead of the exp accumulator -- DVE queuing
delayed the et copies ~2.4us, Pool lacks the accumulator op.)
"""

import sys

import numpy as np

if "/opt/trn_rl_repo" not in sys.path:
    sys.path.insert(0, "/opt/trn_rl_repo")

B = 4
S = 2048
D = 1024
NB = 8  # query blocks of 128 per core
KH = 8  # 128-row tiles along d_in / d_out
NEG = -1.0e9
WSM = 32.0  # host-side scale on M = Wq @ Wk^T (scores come out 32x)
SCALE = float(D) ** -0.5 / WSM  # exp scale absorbs the 32x
_CACHE = {}
LAST_RESULT = None


def _build_nc():
    import contextlib

    import concourse.bacc as bacc
    import concourse.mybir as mybir
    import concourse.tile as tile

    F32 = mybir.dt.float32
    F8 = mybir.dt.float8e4
    BF = mybir.dt.bfloat16
    DR = mybir.MatmulPerfMode.DoubleRow

    nc = bacc.Bacc(None, target_bir_lowering=False)

    # All inputs arrive HOST-TILED: partition dim first and each
    # partition's SBUF content one contiguous DRAM run (>=4KB DMA lines;
    # the natural layouts produced 512-1024B lines that measured ~half of
    # the 358GB/s HBM rate and starved the kernel start). Need-ordered
    # chunks (key-chunks, q-strips) are SEPARATE dram tensors so every
    # DMA is a rank-matched whole/contiguous slice -- slicing a 4D tensor
    # leaves singleton dims in the APs, which cost ~40ns/matmul when the
    # same trick was tried on the SBUF side.
    x_nat = nc.dram_tensor("x_nat", [128, S // 128, D], BF, kind="ExternalInput")
    xT8_c = [
        nc.dram_tensor(f"xT8_{c}", [128, KH, 512], F8, kind="ExternalInput")
        for c in range(4)
    ]
    xq8_s = [
        nc.dram_tensor(f"xq8_{s}", [128, KH, 512], F8, kind="ExternalInput")
        for s in range(2)
    ]
    m8 = nc.dram_tensor("m8", [128, KH, D], F8, kind="ExternalInput")
    wv = nc.dram_tensor("wv", [128, KH, D], BF, kind="ExternalInput")
    mask = nc.dram_tensor("mask", [128, 2, 512], BF, kind="ExternalInput")
    ident_in = nc.dram_tensor("ident", [128, 128], BF, kind="ExternalInput")
    y = nc.dram_tensor("y", [NB * 128, D], F32, kind="ExternalOutput")
    warm_dram = nc.dram_tensor("warm_scratch", [128, 256], BF)  # HAM warm-up

    with tile.TileContext(nc) as tc:
        with contextlib.ExitStack() as ctx:
            persist = ctx.enter_context(tc.tile_pool(name="persist", bufs=1))

            ident = persist.tile([128, 128], BF)
            mask_sb = persist.tile([128, 2, 512], BF)
            # one tile per 512-col strip: engine-write dependencies are
            # tracked whole-tile, so a single t_T tile would gate the first
            # scores on the LAST strip's PSUM->SBUF copies
            t_Ts = [
                persist.tile([128, KH, 512], F8, name="t_T0"),  # [d_lo,d_hi,sq]
                persist.tile([128, KH, 512], F8, name="t_T1"),
            ]
            # per-chunk key tiles keep the scores rhs a clean 3D AP
            xT_tiles = [
                persist.tile([128, KH, 512], F8, name=f"xTk{c}") for c in range(4)
            ]
            x_keep = persist.tile([128, S // 128, D], BF)  # [s_lo, s_hi, d]
            wv_sb = persist.tile([128, KH, D], BF)  # P2 out-projection
            l_acc = persist.tile([128, NB], F32)

            # sps (the phase-2 score PSUM pool) is opened OUTSIDE phase 1 so
            # it owns two banks that phase-1's mmps never touches: PSUM pool
            # bank reuse orders the first scores matmul behind ALL t_T
            # PSUM->SBUF casts otherwise (measured 1.3us gap).
            sps_pool = ctx.enter_context(
                tc.tile_pool(name="sps", bufs=2, space="PSUM")
            )

            # ---------------- Phase 1: t = x_q @ M projection ----------------
            with (
                tc.tile_pool(name="w8pool", bufs=1) as w8pool,
                tc.tile_pool(name="xq8", bufs=2) as xq8_pool,
                tc.tile_pool(name="mmps", bufs=6, space="PSUM") as mmps_pool,
            ):
                # PE warm-up: dependency-free bf16 matmuls on memset data
                # cover the first-DMA dead window (~7.7-11.2us: m8/xq land
                # ~11us) and bring the HAM clock gate to full rate. bf16
                # single-pass (fp32 ran two passes and overshot data-ready
                # by 2us, delaying the projection). A 16-col slice is
                # written out so the chain isn't dead; kept tiny so the
                # PSUM slot's consumers finish right after the last warm
                # matmul.
                # 9 matmuls ~= 4.0us: sustained past the 3.4us HAM window
                # (7 measured 3.17us and the gate never opened -> cold
                # projection) and ending right at first-input-ready ~12.2us
                warm = persist.tile([128, 512], BF)
                nc.vector.memset(warm, 0.0)
                wps = sps_pool.tile([128, 512], F32, tag="s")
                for m in range(8):
                    nc.tensor.matmul(
                        wps, warm[:, :128], warm,
                        start=(m == 0), stop=(m == 7),
                    )
                nc.vector.tensor_copy(out=warm[:, :16], in_=wps[:, :16])
                nc.gpsimd.dma_start(out=warm_dram[:, :16], in_=warm[:, :16])

                # Weight-side DMAs on the scalar (ACT) HWDGE queue; all x
                # DMAs on the sync (SP) queue. Few LARGE descriptors: each
                # DMA trigger costs ~630ns of queue-engine issue time, so
                # batch per-tile (3D APs) -- only the first m8 halves stay
                # split so the t projection can start ASAP.
                # m8 arrives as kh-PAIR slices (256KB each) matching
                # k_outer's kp consumption order: the first matmuls then
                # gate on ~384KB instead of 1MB (early DMA runs at only
                # ~180-250GB/s per queue, so prerequisite size is the
                # kernel-start lever)
                m8_sb = w8pool.tile([128, KH, D], F8, tag="w8")
                for pp in range(4):
                    nc.scalar.dma_start(
                        out=m8_sb[:, 2 * pp : 2 * pp + 2, :],
                        in_=m8[:, 2 * pp : 2 * pp + 2, :],
                    )
                for hh in range(2):
                    nc.scalar.dma_start(
                        out=wv_sb[:, hh * 4 : (hh + 1) * 4, :],
                        in_=wv[:, hh * 4 : (hh + 1) * 4, :],
                    )

                # sync-queue DMAs strictly in first-need order (HBM BW is
                # the startup constraint at ~358GB/s shared across queues):
                # xq strips gate the t projection (~11us), then per-chunk
                # interleave of x_T8 (scores, need ~25.5+2.2j us), mask/
                # ident (first diag/transpose ~25.5us) and x_nat quarters
                # (AV, need ~26.5+2.2j us).
                xq_tiles = []
                for strip in range(2):
                    xTq = xq8_pool.tile([128, KH, 512], F8, tag="xq8")
                    # kh-pair slices (128KB): each kp burst waits only on
                    # its own pair (a whole-strip descriptor stalled the
                    # strip-1 matmuls ~0.9us mid-projection)
                    for pp in range(4):
                        nc.sync.dma_start(
                            out=xTq[:, 2 * pp : 2 * pp + 2, :],
                            in_=xq8_s[strip][:, 2 * pp : 2 * pp + 2, :],
                        )
                    xq_tiles.append(xTq)

                def xT8_chunk(chunk):
                    nc.sync.dma_start(
                        out=xT_tiles[chunk], in_=xT8_c[chunk][:, :, :]
                    )

                def xnat_quarter(q):
                    nc.sync.dma_start(
                        out=x_keep[:, q * 4 : (q + 1) * 4, :],
                        in_=x_nat[:, q * 4 : (q + 1) * 4, :],
                    )

                xT8_chunk(0)
                nc.sync.dma_start(out=mask_sb, in_=mask[:, :, :])
                nc.sync.dma_start(out=ident, in_=ident_in[:, :])
                xnat_quarter(0)
                xT8_chunk(1)
                xnat_quarter(1)
                xT8_chunk(2)
                xnat_quarter(2)
                xT8_chunk(3)
                xnat_quarter(3)

                def t_cast(dst, src, h):
                    # PSUM->SBUF fp8 casts alternate DVE/ScalarE so the
                    # cast train never becomes the critical path when a
                    # PSUM slot is re-used
                    if h % 2 == 0:
                        nc.vector.tensor_copy(out=dst, in_=src)
                    else:
                        nc.scalar.mul(out=dst, in_=src, mul=1.0)

                def t_segment(strip, k_outer=False):
                    xTq = xq_tiles[strip]
                    t_T = t_Ts[strip]
                    if k_outer:
                        # startup: kp-outer over a 6-bank h-group (6 MMs =
                        # 1.3us per kp-pair, matching the ~1.2us pair-DMA
                        # arrival cadence so the PE neither stalls nor
                        # outruns the stream; 6 banks keeps mmps+sps within
                        # PSUM), then a 2-bank tail once all data is in
                        for hg, nh in ((0, 6), (6, 2)):
                            qpss = []
                            for _h in range(nh):
                                qt = mmps_pool.tile([128, 512], F32, tag="mm")
                                qpss.append(qt)
                            for kp in range(KH // 2):
                                for hh in range(nh):
                                    h = hg + hh
                                    nc.tensor.matmul(
                                        qpss[hh],
                                        m8_sb[:, 2 * kp : 2 * kp + 2, h * 128 : (h + 1) * 128],
                                        xTq[:, 2 * kp : 2 * kp + 2, :],
                                        start=(kp == 0),
                                        stop=(kp == KH // 2 - 1),
                                        perf_mode=DR,
                                    )
                            for hh in range(nh):
                                t_cast(t_T[:, hg + hh, :], qpss[hh], hh)
                        return
                    for h in range(KH):
                        qps = mmps_pool.tile([128, 512], F32, tag="mm")
                        for kp in range(KH // 2):
                            nc.tensor.matmul(
                                qps,
                                m8_sb[:, 2 * kp : 2 * kp + 2, h * 128 : (h + 1) * 128],
                                xTq[:, 2 * kp : 2 * kp + 2, :],
                                start=(kp == 0),
                                stop=(kp == KH // 2 - 1),
                                perf_mode=DR,
                            )
                        t_cast(t_T[:, h, :], qps, h)

                t_segment(0, k_outer=True)
                t_segment(1)

            # ---------------- Phase 2: attention ----------------
            # i-major: each query block i accumulates AV over all its key
            # blocks j=0..i//2 in ONE long PSUM group (no SBUF out_acc at
            # all); the finalize is a single fused (avps * 1/l) PSUM->SBUF
            # op. The PE stream is software-pipelined one j ahead: scores
            # for j+1 are emitted before transposes/AV of j, so the PE
            # never waits on the scalar exp except at the very tail.
            with (
                tc.tile_pool(name="esb", bufs=3) as esb_pool,
                tc.tile_pool(name="etsb", bufs=3) as etsb_pool,
                tc.tile_pool(name="lsb", bufs=4) as lsb_pool,
                tc.tile_pool(name="ysb", bufs=2) as ysb_pool,
                tc.tile_pool(name="etps", bufs=2, space="PSUM") as etps_pool,
                tc.tile_pool(name="avps", bufs=2, space="PSUM") as avps_pool,
                tc.tile_pool(name="outps", bufs=2, space="PSUM") as outps_pool,
                tc.tile_pool(name="usb", bufs=2) as usb_pool,
            ):
                def emit_scores(i, j, diag, ncols):
                    t_T = t_Ts[i // 4]
                    icol = (i % 4) * 128
                    sps = sps_pool.tile([128, 512], F32, tag="s")
                    for kp in range(KH // 2):
                        nc.tensor.matmul(
                            sps[:, :ncols],
                            t_T[:, 2 * kp : 2 * kp + 2, icol : icol + 128],
                            xT_tiles[j][:, 2 * kp : 2 * kp + 2, :ncols],
                            start=(kp == 0),
                            stop=(kp == KH // 2 - 1),
                            perf_mode=DR,
                        )
                    if diag:
                        # the causal boundary only touches a 256-wide slice:
                        # even i -> cols [0:256) of m0; odd i -> [256:512)
                        # of m1 (cols [0:256) are always fully visible)
                        lo = 0 if i % 2 == 0 else 256
                        nc.vector.tensor_add(
                            out=sps[:, lo : lo + 256],
                            in0=sps[:, lo : lo + 256],
                            in1=mask_sb[:, i % 2, lo : lo + 256],
                        )
                    # l rides the exp's free-axis accumulator: the extra
                    # ACTIVATION_READ_ACCUMULATOR (~280ns) costs less than
                    # any alternative engine for the reduction (DVE pass
                    # delays the et copies ~2.4us across the kernel; Pool
                    # lacks the accumulator op)
                    e_sb = esb_pool.tile([128, 512], BF, tag="e")
                    lpart = lsb_pool.tile([128, 1], F32, tag="l")
                    nc.scalar.activation(
                        out=e_sb[:, :ncols],
                        in_=sps[:, :ncols],
                        func=mybir.ActivationFunctionType.Exp,
                        scale=SCALE,
                        accum_out=lpart,
                    )
                    if j == 0:
                        nc.vector.tensor_copy(out=l_acc[:, i : i + 1], in_=lpart)
                    else:
                        nc.vector.tensor_add(
                            out=l_acc[:, i : i + 1],
                            in0=l_acc[:, i : i + 1],
                            in1=lpart,
                        )
                    return e_sb

                def emit_av(i, j, diag, ncols, e_sb, av, jmax):
                    njj = ncols // 128
                    etp = etps_pool.tile([128, 1024], BF, tag="et")
                    for jj in range(njj):
                        nc.tensor.transpose(
                            etp[:, jj * 128 : (jj + 1) * 128],
                            e_sb[:, jj * 128 : (jj + 1) * 128],
                            ident,
                        )
                    et = etsb_pool.tile([128, 512], BF, tag="ets")
                    nc.vector.tensor_copy(out=et[:, :ncols], in_=etp[:, :ncols])
                    # dh-major with separate per-bank PSUM tiles: each half
                    # finishes accumulating independently so the finalize
                    # of half 0 overlaps the PE work on half 1
                    for dh in range(2):
                        for jj in range(njj):
                            nc.tensor.matmul(
                                av[dh],
                                et[:, jj * 128 : (jj + 1) * 128],
                                x_keep[:, 4 * j + jj, dh * 512 : (dh + 1) * 512],
                                start=(j == 0 and jj == 0),
                                stop=(j == jmax and jj == njj - 1),
                                skip_group_check=True,
                            )

                def finalize_a(i, av):
                    # U = E@X accumulated in PSUM: copy out (split across
                    # ScalarE + DVE) and transpose on the PE. The 1/l
                    # scale rides the final PSUM->SBUF copy in finalize_b,
                    # so nothing here waits on the diag exp / reciprocal.
                    # asymmetric ScalarE/DVE splits: the first consumer
                    # (transpose b0 / out-proj kd0) gates only on the short
                    # 256-col ScalarE piece (~345ns) while the DVE piece
                    # finishes under the already-running PE work
                    ustage = usb_pool.tile([128, D], BF, tag="u")
                    nc.scalar.mul(out=ustage[:, 0:256], in_=av[0][:, 0:256], mul=1.0)
                    nc.vector.tensor_copy(out=ustage[:, 256:512], in_=av[0][:, 256:512])
                    nc.vector.tensor_copy(out=ustage[:, 512:1024], in_=av[1])
                    utp = etps_pool.tile([128, 1024], BF, tag="et")
                    for b in range(KH):
                        nc.tensor.transpose(
                            utp[:, b * 128 : (b + 1) * 128],
                            ustage[:, b * 128 : (b + 1) * 128],
                            ident,
                        )
                    ut = usb_pool.tile([128, D], BF, tag="ut")
                    nc.scalar.mul(out=ut[:, 0:256], in_=utp[:, 0:256], mul=1.0)
                    nc.vector.tensor_copy(out=ut[:, 256:D], in_=utp[:, 256:D])
                    return ut

                def finalize_b(i, ut, rinv):
                    # out = (U @ Wv) / l. Deferred one pipeline pop after
                    # finalize_a so the next stage's scores+AV cover the
                    # ut PSUM->SBUF copy latency (measured ~0.8us exposed
                    # when the out-projection followed it immediately).
                    ystage = ysb_pool.tile([128, D], F32, tag="y")
                    for dh in range(2):
                        ops = outps_pool.tile([128, 512], F32, tag="o")
                        for kd in range(KH):
                            nc.tensor.matmul(
                                ops,
                                ut[:, kd * 128 : (kd + 1) * 128],
                                wv_sb[:, kd, dh * 512 : (dh + 1) * 512],
                                start=(kd == 0),
                                stop=(kd == KH - 1),
                            )
                        nc.scalar.mul(
                            out=ystage[:, dh * 512 : (dh + 1) * 512],
                            in_=ops,
                            mul=rinv,
                        )
                        # sync HWDGE: faster end-of-kernel drain than SWDGE
                        nc.sync.dma_start(
                            out=y[i * 128 : (i + 1) * 128, dh * 512 : (dh + 1) * 512],
                            in_=ystage[:, dh * 512 : (dh + 1) * 512],
                        )

                # global one-ahead pipeline ACROSS i boundaries: the scores
                # of the next (i, j) stage are always emitted before the
                # transposes+AV of the previous stage, so the PE never
                # waits on the scalar exp -- even through single-j blocks
                # 2 first: its opening stage is non-diag, so the first
                # exp chain has no mask-add latency to cover; 6 last:
                # shortest finalize tail
                order = (2, 0, 1, 3, 4, 5, 7, 6)
                stages = [
                    (i, j, i // 2) for i in order for j in range(i // 2 + 1)
                ]
                avs = {}
                rinvs = {}
                fin_pending = []  # at most one deferred (i, ut, rinv)

                def process(p):
                    pi, pj, pdiag, pncols, pe_sb, pjmax = p
                    emit_av(pi, pj, pdiag, pncols, pe_sb, avs[pi], pjmax)
                    if fin_pending:
                        finalize_b(*fin_pending.pop())
                    if pj == pjmax:
                        # l complete once the diag lpart lands; 1/l runs
                        # while the PE does transposes + out-projection
                        rinv = lsb_pool.tile([128, 1], F32, tag="r")
                        nc.vector.reciprocal(out=rinv, in_=l_acc[:, pi : pi + 1])
                        rinvs[pi] = rinv
                        ut = finalize_a(pi, avs[pi])
                        fin_pending.append((pi, ut, rinv))

                # depth-2: scores run TWO stages ahead of transposes+AV, so
                # every exp chain has ~1.7us of PE cover (one stage left
                # the first exp + the diag chains ~0.5-1.9us exposed and
                # occasionally idled the PE into a HAM re-throttle). sps
                # bufs=2 still suffices: exp(n) consumes its score PSUM
                # during scores(n+1), before scores(n+2) reuses the slot.
                pq = []
                for i, j, jmax in stages:
                    if j == 0:
                        av0 = avps_pool.tile([128, 512], F32, tag="av")
                        av1 = avps_pool.tile([128, 512], F32, tag="av")
                        avs[i] = [av0, av1]
                    diag = j == jmax
                    ncols = 256 if (diag and i % 2 == 0) else 512
                    e_sb = emit_scores(i, j, diag, ncols)
                    pq.append((i, j, diag, ncols, e_sb, jmax))
                    if len(pq) > 2:
                        process(pq.pop(0))
                for p in pq:
                    process(p)
                if fin_pending:
                    finalize_b(*fin_pending.pop())

    return nc


def _get_nc(finalize=True):
    key = "nc_fin" if finalize else "nc_raw"
    if key not in _CACHE:
        nc = _build_nc()
        if finalize:
            nc.finalize()
        _CACHE[key] = nc
    return _CACHE[key]


def make_in_maps(x, Wq, Wk, Wv):
    """All tensors are host-tiled so every SBUF partition's content is one
    contiguous DRAM run (2-8KB DMA lines; natural layouts gave 512-1024B
    lines at ~half HBM rate), with free-dim chunk granularity outermost."""
    import ml_dtypes

    f8 = ml_dtypes.float8_e4m3
    bf = ml_dtypes.bfloat16
    ident = np.eye(128, dtype=np.float32).astype(bf)
    p = np.arange(128)[:, None]
    c = np.arange(512)[None, :]
    M = np.asarray(Wq, dtype=np.float64) @ np.asarray(Wk, dtype=np.float64).T
    m8_flat = np.ascontiguousarray(M * WSM).astype(np.float32).astype(f8)
    # [d_in, d_out] -> [p=d_in%128, kh=d_in//128, d_out]
    m8_np = np.ascontiguousarray(m8_flat.reshape(KH, 128, D).transpose(1, 0, 2))
    wv_flat = np.ascontiguousarray(Wv, dtype=np.float32).astype(bf)
    wv_np = np.ascontiguousarray(wv_flat.reshape(KH, 128, D).transpose(1, 0, 2))
    in_maps = []
    for core in range(8):
        b, par = core // 2, core % 2
        # mask[0]: boundary block for even local i; mask[1]: odd local i
        m0 = np.where(c <= p + par * 128, 0.0, NEG).astype(bf)
        m1 = np.where(c <= 256 + par * 128 + p, 0.0, NEG).astype(bf)
        mask_np = np.ascontiguousarray(np.stack([m0, m1]).transpose(1, 0, 2))
        xb = np.asarray(x[b], dtype=np.float32)
        xb8 = xb.astype(f8)
        xq8 = xb8.reshape(16, 128, D)[par::2].reshape(NB * 128, D)
        # x^T [d, s] -> per key-chunk c: [p=d%128, kh=d//128, s%512]
        xT8_t = xb8.T.reshape(KH, 128, 4, 512).transpose(2, 1, 0, 3)
        # x_q^T [d, q] -> per strip s: [p, kh, q%512]
        xq8_t = xq8.T.reshape(KH, 128, 2, 512).transpose(2, 1, 0, 3)
        # x [s, d] -> [p=s%128, s_hi=s//128, d]
        xnat_np = np.ascontiguousarray(
            xb.astype(bf).reshape(16, 128, D).transpose(1, 0, 2)
        )
        im = {
            "x_nat": xnat_np,
            "m8": m8_np,
            "wv": wv_np,
            "mask": mask_np,
            "ident": ident,
        }
        for cc in range(4):
            im[f"xT8_{cc}"] = np.ascontiguousarray(xT8_t[cc])
        for ss in range(2):
            im[f"xq8_{ss}"] = np.ascontiguousarray(xq8_t[ss])
        in_maps.append(im)
    return in_maps


def assemble_out(results):
    out = np.empty((B, S, D), dtype=np.float32)
    o4 = out.reshape(B, 16, 128, D)
    for core in range(8):
        b, par = core // 2, core % 2
        o4[b, par::2] = results[core]["y"].reshape(NB, 128, D)
    return out


def _ensure_axon_hooks_shim():
    """bass_utils imports antenv.axon_hooks when BASS_TRACE is set; provide a
    no-op fallback so a stray BASS_TRACE env var can't crash the run."""
    try:
        import antenv.axon_hooks  # noqa: F401
    except ImportError:
        import types

        import antenv

        mod = types.ModuleType("antenv.axon_hooks")
        mod.get_axon_ntff_profile_hook = lambda: None
        mod.set_axon_ntff_profile_hook = lambda h: None
        sys.modules["antenv.axon_hooks"] = mod
        antenv.axon_hooks = mod


def kernel(x, Wq, Wk, Wv):
    global LAST_RESULT
    from concourse.bass_utils import run_bass_kernel_spmd

    _ensure_axon_hooks_shim()
    nc = _get_nc(finalize=True)
    in_maps = make_in_maps(x, Wq, Wk, Wv)
    res = run_bass_kernel_spmd(nc, in_maps, core_ids=list(range(8)))
    LAST_RESULT = res
    return assemble_out(res.results)
